# revision 43
# baseline (speedup 1.0000x reference)
"""Trainium2 Bass kernel for DSQG attention (J=12 causal-offset sparse attention).

Sharding: data-parallel over (B,H): 32 bh-slices -> 8 cores x 4 bh.
Each core processes its 4 bh as 2 stacked pairs s in a transposed layout
[128 = 2bh x 64hd, N] so every sequence shift is a free-dim AP offset.

v3 (bf16 overhaul, 519us -> 221us cost-model):
  - all data bf16: PE matmuls 1 cyc/row (vs 4 for fp32), DVE TT 2x mode,
    half the HBM traffic. rel err 6.9e-3 (tol 2e-2).
  - unified score layout: score/e row = 32*a + 2*g + bh (i = 4g+a) so all
    12 offsets live in ONE [128, 512] PSUM tile -> 1 exp ACT op per half,
    single-matmul denominator / rot-broadcast / e-broadcast selectors.
  - score products k_sh*q split gpsimd/DVE (16/8 of 24 halves; walrus
    rejects STT on Pool, so +q*se_i rides a seG matmul into the same PSUM
    accumulation group).
  - [D]: PE broadcasts e_i (bsel), ACT copies PSUM->SBUF bf16, DVE does
    the e*v muls in 2x mode, and PE identity-matmuls accumulate the 12
    products in PSUM (replaces the 11-deep DVE add chain); the rotation
    correction (rotred) accumulates into the same PSUM group.
  - theta/sin computed once for BOTH stacks (128 compact rows = 2s x 2bh
    x 8i x 2p x 2cs) in a pre-pass: ACT loads the trig table once, exp
    table once (baseline reloaded tables 16x); cos-1 pre-folded into
    trig_full via a per-row scalar add; one range-wrap (|theta|<3pi).
  - DMA order tuned: chunk-0 q/k slices first, zero-padding memset
    on-chip instead of DMA'd, outputs bf16.
"""

import sys

for _p in ("/opt/trn_rl_repo", "/root/.axon_site/_ro/trn_rl_repo"):
    if _p not in sys.path:
        sys.path.insert(0, _p)

import numpy as np

OFFSETS = (1, 2, 4, 8, 16, 64, 96, 192, 384, 512, 768, 1024)
J = 12
B, H, N, HD = 2, 16, 4096, 64
PAD = 1024
NP_ = N + PAD
CH = 1024            # chunk width for SBUF/elementwise ops
CHA = 512            # PSUM-facing sub-chunk (one bank)
NCHUNK = N // CH
SC = 1.0 / 8.0
NCORES = 8
ROT = OFFSETS[4:]    # 8 rotating offsets (abs i = 4..11)
T_P = (0, 0, 0, 0, 1, 1, 1, 1)      # phase pair per term slot t
T_CH = (0, 1, 0, 1, 2, 3, 2, 3)     # v channel per t
T_CS = (0, 0, 1, 1, 0, 0, 1, 1)     # 0 = cos branch, 1 = sin branch

# score/e row for offset i = 4g + a (a = i%4, g = i//4), local bh in {0,1}
def _rsc(i, bh):
    a, g = i % 4, i // 4
    return 32 * a + 2 * g + bh

# rot-stack row (per s)
def _rrot(bh, i8, t):
    return 64 * bh + 8 * i8 + t

# trig compact row (shared over both s)
def _rtc(s, bh, i8, p, cs):
    return 64 * s + 32 * bh + 4 * i8 + 2 * p + cs

# engine assignment knobs
POOL_NPROD = 16                 # of 24 half-products per (c,s) on gpsimd
POOL_NPROD_RAMP = 10            # chunk-0 split (DVE idle during ramp)
ACT_COPY_D = set(range(12))     # offsets whose e-broadcast is ACT-copied to bf16
W_COPIES = ("trig8", "erps", "rotps", "rbps")   # extra PSUM->bf16 ACT copies

_PROGRAM = None


def _build_program():
    import concourse.tile as tile
    from concourse import bacc, mybir

    f32 = mybir.dt.float32
    bf16 = mybir.dt.bfloat16
    AluOp = mybir.AluOpType
    Act = mybir.ActivationFunctionType

    nc = bacc.Bacc()
    dp = nc.declare_dram_parameter

    ins = {}
    for s in range(2):
        ins[f"qT{s}"] = dp(f"qT{s}", [128, N], bf16, isOutput=False)
        ins[f"kTp{s}"] = dp(f"kTp{s}", [128, N], bf16, isOutput=False)
        ins[f"vTp{s}"] = dp(f"vTp{s}", [128, N], bf16, isOutput=False)
        ins[f"vsh{s}"] = dp(f"vsh{s}", [128, N], bf16, isOutput=False)
        ins[f"pbc{s}"] = dp(f"pbc{s}", [128, 1], f32, isOutput=False)
    ins["y128"] = dp("y128", [128, N], bf16, isOutput=False)
    ins["z128"] = dp("z128", [128, N], bf16, isOutput=False)
    ins["g128"] = dp("g128", [128, 1], f32, isOutput=False)
    ins["b128"] = dp("b128", [128, 1], f32, isOutput=False)
    ins["onesG"] = dp("onesG", [128, 3 * 32], bf16, isOutput=False)
    ins["esel"] = dp("esel", [128, 2], bf16, isOutput=False)
    ins["bsel"] = dp("bsel", [128, J * 128], bf16, isOutput=False)
    ins["rotsel"] = dp("rotsel", [128, 128], bf16, isOutput=False)
    ins["rotred"] = dp("rotred", [128, 128], bf16, isOutput=False)
    ins["rsel"] = dp("rsel", [64, 2 * 128], bf16, isOutput=False)
    ins["texp"] = dp("texp", [128, 2 * 128], bf16, isOutput=False)
    ins["cm1"] = dp("cm1", [128, 1], f32, isOutput=False)
    ins["ident"] = dp("ident", [128, 128], bf16, isOutput=False)
    ins["cmask"] = dp("cmask", [128, CH], bf16, isOutput=False)
    ins["seG"] = dp("seG", [128, 128], bf16, isOutput=False)
    outs = [dp(f"outT{s}", [128, N], bf16, isOutput=True) for s in range(2)]

    with tile.TileContext(nc) as tc:
        with (
            nc.allow_low_precision(
                reason="bf16 weighted-value accumulation; rel-tol 2e-2"),
            tc.tile_pool(name="consts", bufs=1) as cpool,
            tc.tile_pool(name="data", bufs=1) as dpool,
            tc.tile_pool(name="work", bufs=2) as wpool,
            tc.tile_pool(name="ecp", bufs=6) as ecpool,
            tc.tile_pool(name="prods", bufs=24) as ppool,
            tc.tile_pool(name="bcast", bufs=4) as bpool,
            tc.tile_pool(name="tmpp", bufs=3) as tpool,
            tc.tile_pool(name="mmp", bufs=12) as mmpool,
            tc.tile_pool(name="thp", bufs=1) as thpool,
            tc.tile_pool(name="psA", bufs=1, space="PSUM") as psA,   # scps
            tc.tile_pool(name="psB", bufs=1, space="PSUM") as psB,   # e-bcast
            tc.tile_pool(name="psACC", bufs=1, space="PSUM") as psACC,  # acc
            tc.tile_pool(name="psC", bufs=2, space="PSUM") as psC,   # rot seq
            tc.tile_pool(name="psD", bufs=1, space="PSUM") as psD,   # den/rb
        ):
            # DMA order tuned for ramp: chunk-0 score path first.
            qT = [None, None]
            kTp = [None, None]
            vTp = [None, None]
            vsh = [None, None]
            c_pbc = [None, None]
            qT[0] = dpool.tile([128, N], bf16, tag="qT0", name="qT0")
            nc.sync.dma_start(out=qT[0][:, 0:CH], in_=ins["qT0"][:, 0:CH])
            kTp[0] = dpool.tile([128, NP_], bf16, tag="kTp0", name="kTp0")
            nc.gpsimd.memset(kTp[0][:, 0:PAD], 0.0)
            nc.sync.dma_start(out=kTp[0][:, PAD:PAD + CH],
                              in_=ins["kTp0"][:, 0:CH])
            c_onesG = cpool.tile([128, 3 * 32], bf16, tag="c_onesG")
            nc.sync.dma_start(out=c_onesG, in_=ins["onesG"][:])
            c_seG = cpool.tile([128, 128], bf16, tag="c_seG")
            nc.sync.dma_start(out=c_seG, in_=ins["seG"][:])
            c_pbc[0] = cpool.tile([128, 1], f32, tag="c_pbc0", name="c_pbc0")
            nc.sync.dma_start(out=c_pbc[0], in_=ins["pbc0"][:])
            c_cmask = cpool.tile([128, CH], bf16, tag="c_cmask")
            nc.sync.dma_start(out=c_cmask, in_=ins["cmask"][:])
            nc.sync.dma_start(out=qT[0][:, CH:], in_=ins["qT0"][:, CH:])
            nc.sync.dma_start(out=kTp[0][:, PAD + CH:],
                              in_=ins["kTp0"][:, CH:])
            qT[1] = dpool.tile([128, N], bf16, tag="qT1", name="qT1")
            nc.sync.dma_start(out=qT[1], in_=ins["qT1"][:])
            kTp[1] = dpool.tile([128, NP_], bf16, tag="kTp1", name="kTp1")
            nc.gpsimd.memset(kTp[1][:, 0:PAD], 0.0)
            nc.sync.dma_start(out=kTp[1][:, PAD:], in_=ins["kTp1"][:])
            c_pbc[1] = cpool.tile([128, 1], f32, tag="c_pbc1", name="c_pbc1")
            nc.sync.dma_start(out=c_pbc[1], in_=ins["pbc1"][:])
            c_esel = cpool.tile([128, 2], bf16, tag="c_esel")
            nc.sync.dma_start(out=c_esel, in_=ins["esel"][:])
            c_bsel = cpool.tile([128, J * 128], bf16, tag="c_bsel")
            nc.sync.dma_start(out=c_bsel, in_=ins["bsel"][:])
            c_ident = cpool.tile([128, 128], bf16, tag="c_ident")
            nc.sync.dma_start(out=c_ident, in_=ins["ident"][:])
            vTp[0] = dpool.tile([128, NP_], bf16, tag="vTp0", name="vTp0")
            nc.gpsimd.memset(vTp[0][:, 0:PAD], 0.0)
            nc.sync.dma_start(out=vTp[0][:, PAD:], in_=ins["vTp0"][:])
            vTp[1] = dpool.tile([128, NP_], bf16, tag="vTp1", name="vTp1")
            nc.gpsimd.memset(vTp[1][:, 0:PAD], 0.0)
            nc.sync.dma_start(out=vTp[1][:, PAD:], in_=ins["vTp1"][:])
            y128 = dpool.tile([128, N], bf16, tag="y128")
            nc.sync.dma_start(out=y128, in_=ins["y128"][:])
            z128 = dpool.tile([128, N], bf16, tag="z128")
            nc.sync.dma_start(out=z128, in_=ins["z128"][:])
            c_g128 = cpool.tile([128, 1], f32, tag="c_g128")
            nc.sync.dma_start(out=c_g128, in_=ins["g128"][:])
            c_b128 = cpool.tile([128, 1], f32, tag="c_b128")
            nc.sync.dma_start(out=c_b128, in_=ins["b128"][:])
            vsh[0] = dpool.tile([128, N], bf16, tag="vsh0", name="vsh0")
            nc.sync.dma_start(out=vsh[0], in_=ins["vsh0"][:])
            c_rotsel = cpool.tile([128, 128], bf16, tag="c_rotsel")
            nc.sync.dma_start(out=c_rotsel, in_=ins["rotsel"][:])
            c_rotred = cpool.tile([128, 128], bf16, tag="c_rotred")
            nc.sync.dma_start(out=c_rotred, in_=ins["rotred"][:])
            c_rsel = cpool.tile([64, 2 * 128], bf16, tag="c_rsel")
            nc.sync.dma_start(out=c_rsel, in_=ins["rsel"][:])
            c_texp = cpool.tile([128, 2 * 128], bf16, tag="c_texp")
            nc.sync.dma_start(out=c_texp, in_=ins["texp"][:])
            c_cm1 = cpool.tile([128, 1], f32, tag="c_cm1")
            nc.sync.dma_start(out=c_cm1, in_=ins["cm1"][:])
            vsh[1] = dpool.tile([128, N], bf16, tag="vsh1", name="vsh1")
            nc.sync.dma_start(out=vsh[1], in_=ins["vsh1"][:])

            # ---------- trig pre-pass: trig_full for BOTH s ----------
            trig_full = dpool.tile([128, N], bf16, tag="trig_full")
            for pc in range(NCHUNK):
                n0 = pc * CH
                ub = wpool.tile([128, CH], bf16, tag="ub")
                nc.vector.tensor_mul(ub, y128[:, n0:n0 + CH],
                                     z128[:, n0:n0 + CH])
                th = thpool.tile([128, CH], f32, tag="th")
                nc.vector.tensor_scalar(
                    out=th, in0=ub,
                    scalar1=c_g128[:, 0:1], scalar2=c_b128[:, 0:1],
                    op0=AluOp.mult, op1=AluOp.add,
                )
                nc.vector.add_range_wrap(th, th, 0.0, np.pi, 2.0 * np.pi)
                trg = wpool.tile([128, CH], bf16, tag="trg")
                nc.scalar.activation(out=trg, in_=th,
                                     func=Act.Sin, bias=0.0, scale=1.0)
                nc.vector.tensor_scalar_add(trig_full[:, n0:n0 + CH],
                                            trg, c_cm1[:, 0:1])

            def phase_A(c):
                n0 = c * CH
                # ---------- [A] scores + exp (per s) ----------
                nprod = POOL_NPROD_RAMP if c == 0 else POOL_NPROD
                ec = [None, None]
                for s in range(2):
                    ec[s] = ecpool.tile([128, CH], bf16, tag="ec",
                                        name=f"ec{s}")
                    for hf in range(2):
                        h0 = hf * CHA
                        scps = psA.tile([128, CHA], f32, tag="scps")
                        for i, d in enumerate(OFFSETS):
                            a, g = i % 4, i // 4
                            eng = (nc.gpsimd if (2 * i + hf) % 24 < nprod
                                   else nc.vector)
                            pr = ppool.tile([128, CHA], bf16, tag="pr")
                            eng.tensor_mul(
                                pr,
                                kTp[s][:, PAD - d + n0 + h0:
                                       PAD - d + n0 + h0 + CHA],
                                qT[s][:, n0 + h0:n0 + h0 + CHA],
                            )
                            nc.tensor.matmul(
                                out=scps[32 * a:32 * a + 32, :],
                                lhsT=c_onesG[:, 32 * g:32 * g + 32],
                                rhs=pr,
                                start=(g == 0), stop=False,
                                tile_position=(0, 32 * a),
                            )
                        nc.tensor.matmul(
                            out=scps,
                            lhsT=c_seG,
                            rhs=qT[s][:, n0 + h0:n0 + h0 + CHA],
                            start=False, stop=True,
                        )
                        nc.scalar.activation(
                            out=ec[s][:, h0:h0 + CHA], in_=scps,
                            func=Act.Exp, bias=c_pbc[s][:, 0:1], scale=SC,
                        )
                    if c == 0:
                        nc.vector.tensor_mul(ec[s], ec[s], c_cmask)

                # ---------- denominators (both s share recip) ----------
                rcb = wpool.tile([34, CH], bf16, tag="rcb")
                for hf in range(2):
                    h0 = hf * CHA
                    denps = psD.tile([128, CHA], f32, tag="dn", name="denps")
                    for s in range(2):
                        nc.tensor.matmul(
                            out=denps[32 * s:32 * s + 2, :],
                            lhsT=c_esel,
                            rhs=ec[s][:, h0:h0 + CHA],
                            start=True, stop=True,
                        )
                    rf = tpool.tile([34, CHA], f32, tag="rf")
                    nc.vector.tensor_scalar_add(rf, denps[0:34, :], 1e-30)
                    nc.vector.reciprocal(rcb[:, h0:h0 + CHA], rf)
                return ec, rcb

            def phase_B(c, ec, rcb):
                n0 = c * CH
                for s in range(2):
                    accps = psACC.tile([128, CH], f32, tag="accps")
                    # ---------- [D] e-broadcast + weighted v ----------
                    mms = []
                    for i, d in enumerate(OFFSETS):
                        bps = psB.tile([128, CH], f32, tag="bps")
                        for hf in range(2):
                            h0 = hf * CHA
                            nc.tensor.matmul(
                                out=bps[:, h0:h0 + CHA],
                                lhsT=c_bsel[:, i * 128:i * 128 + 128],
                                rhs=ec[s][:, h0:h0 + CHA],
                                start=True, stop=True,
                            )
                        vsl = vTp[s][:, PAD - d + n0: PAD - d + n0 + CH]
                        mm = mmpool.tile([128, CH], bf16, tag="mm")
                        if i in ACT_COPY_D:
                            bcb = bpool.tile([128, CH], bf16, tag="bcb")
                            nc.scalar.activation(out=bcb, in_=bps,
                                                 func=Act.Copy, bias=0.0,
                                                 scale=1.0)
                            nc.vector.tensor_mul(mm, bcb, vsl)
                        else:
                            nc.vector.tensor_mul(mm, bps, vsl)
                        mms.append(mm)
                    for i, mm in enumerate(mms):
                        for hf in range(2):
                            h0 = hf * CHA
                            nc.tensor.matmul(
                                out=accps[:, h0:h0 + CHA],
                                lhsT=c_ident,
                                rhs=mm[:, h0:h0 + CHA],
                                start=(i == 0), stop=False,
                            )

                    # ---------- [R] rotation correction (into accps) ----
                    for hf in range(2):
                        h0 = hf * CHA
                        t8ps = psC.tile([128, CHA], f32, tag="rps",
                                        name="t8ps")
                        nc.tensor.matmul(
                            out=t8ps,
                            lhsT=c_texp[:, s * 128:s * 128 + 128],
                            rhs=trig_full[:, n0 + h0:n0 + h0 + CHA],
                            start=True, stop=True,
                        )
                        vss = vsh[s][:, n0 + h0:n0 + h0 + CHA]
                        prot = tpool.tile([128, CHA], bf16, tag="prot")
                        nc.vector.tensor_mul(prot, t8ps, vss)
                        erps = psC.tile([128, CHA], f32, tag="rps",
                                        name="erps")
                        nc.tensor.matmul(
                            out=erps,
                            lhsT=c_rotsel,
                            rhs=ec[s][:, h0:h0 + CHA],
                            start=True, stop=True,
                        )
                        vful = tpool.tile([128, CHA], bf16, tag="vful")
                        nc.vector.tensor_mul(vful, erps, prot)
                        nc.tensor.matmul(
                            out=accps[:, h0:h0 + CHA],
                            lhsT=c_rotred,
                            rhs=vful,
                            start=False, stop=True,
                        )

                    # ---------- [E] normalize + store ----------
                    outcb = wpool.tile([128, CH], bf16, tag="outcb")
                    for hf in range(2):
                        h0 = hf * CHA
                        rbps = psC.tile([128, CHA], f32, tag="rps",
                                        name="rbps")
                        nc.tensor.matmul(
                            out=rbps,
                            lhsT=c_rsel[32 * s:32 * s + 2, s * 128:s * 128 + 128],
                            rhs=rcb[32 * s:32 * s + 2, h0:h0 + CHA],
                            start=True, stop=True,
                        )
                        rbb = bpool.tile([128, CHA], bf16, tag="rbb")
                        nc.scalar.activation(out=rbb, in_=rbps,
                                             func=Act.Copy, bias=0.0,
                                             scale=1.0)
                        nc.vector.tensor_mul(outcb[:, h0:h0 + CHA],
                                             accps[:, h0:h0 + CHA], rbb)
                    nc.sync.dma_start(out=outs[s][:, n0:n0 + CH], in_=outcb)

            for c in range(NCHUNK):
                ec_c, rcb_c = phase_A(c)
                phase_B(c, ec_c, rcb_c)

    nc.compile()
    return nc


def get_program():
    global _PROGRAM
    if _PROGRAM is None:
        _PROGRAM = _build_program()
    return _PROGRAM


def _shift_np(x, d):
    """out[n] = x[n-d], zeros for n < d; shift along axis 0."""
    out = np.zeros_like(x)
    out[d:] = x[:-d] if d > 0 else x
    return out


def _bf16(x):
    import ml_dtypes
    return np.asarray(x, dtype=np.float32).astype(ml_dtypes.bfloat16)


def _shared_consts(se_global):
    c = {}
    onesG = np.zeros((128, 3 * 32), np.float32)
    for g in range(3):
        for bh in range(2):
            onesG[64 * bh:64 * bh + 64, 32 * g + 2 * g + bh] = 1.0
    c["onesG"] = _bf16(onesG)
    esel = np.zeros((128, 2), np.float32)
    for i in range(J):
        for bh in range(2):
            esel[_rsc(i, bh), bh] = 1.0
    c["esel"] = _bf16(esel)
    bsel = np.zeros((128, J * 128), np.float32)
    for i in range(J):
        for bh in range(2):
            for hd in range(64):
                bsel[_rsc(i, bh), i * 128 + 64 * bh + hd] = 1.0
    c["bsel"] = _bf16(bsel)
    rotsel = np.zeros((128, 128), np.float32)
    for bh in range(2):
        for i8 in range(8):
            abs_i = i8 + 4
            for t in range(8):
                rotsel[_rsc(abs_i, bh), _rrot(bh, i8, t)] = 1.0
    c["rotsel"] = _bf16(rotsel)
    rotred = np.zeros((128, 128), np.float32)
    sign_map = {0: ((0, 1.0), (3, -1.0)), 1: ((1, 1.0), (2, 1.0)),
                2: ((4, 1.0), (7, -1.0)), 3: ((5, 1.0), (6, 1.0))}
    for bh in range(2):
        for ch in range(4):
            col = bh * 64 + ch
            for i8 in range(8):
                for t, sgn in sign_map[ch]:
                    rotred[_rrot(bh, i8, t), col] = sgn
    c["rotred"] = _bf16(rotred)
    rsel = np.zeros((64, 2 * 128), np.float32)
    for s in range(2):
        for bh in range(2):
            rsel[32 * s + bh, s * 128 + 64 * bh: s * 128 + 64 * bh + 64] = 1.0
    c["rsel"] = _bf16(rsel)
    texp = np.zeros((128, 2 * 128), np.float32)
    for s in range(2):
        for bh in range(2):
            for i8 in range(8):
                for p in range(2):
                    for cs in range(2):
                        src = _rtc(s, bh, i8, p, cs)
                        for c2 in range(2):
                            t = 4 * p + 2 * cs + c2
                            texp[src, s * 128 + _rrot(bh, i8, t)] = 1.0
    c["texp"] = _bf16(texp)
    cm1 = np.zeros((128, 1), np.float32)
    for s in range(2):
        for bh in range(2):
            for i8 in range(8):
                for p in range(2):
                    cm1[_rtc(s, bh, i8, p, 0), 0] = -1.0
    c["cm1"] = cm1
    c["ident"] = _bf16(np.eye(128, dtype=np.float32))
    seG = np.zeros((128, 128), np.float32)
    for i in range(J):
        for bh in range(2):
            for hd in range(64):
                seG[64 * bh + hd, _rsc(i, bh)] = se_global[i, hd]
    c["seG"] = _bf16(seG)
    cmask = np.ones((128, CH), np.float32)
    for i, d in enumerate(OFFSETS):
        r = _rsc(i, 0)
        cmask[r:r + 2, 0:d] = 0.0
    c["cmask"] = _bf16(cmask)
    return c


def _core_inputs(core, q, k, v, pb, se, phase_base, phase_gain, y_pre, z_pre,
                 shared):
    m = dict(shared)
    y128 = np.zeros((128, N), np.float32)
    z128 = np.zeros((128, N), np.float32)
    g128 = np.zeros((128, 1), np.float32)
    b128 = np.zeros((128, 1), np.float32)
    for s in range(2):
        bhs = [4 * core + 2 * s, 4 * core + 2 * s + 1]
        qT = np.zeros((128, N), np.float32)
        kTp = np.zeros((128, N), np.float32)
        vTp = np.zeros((128, N), np.float32)
        vsh = np.zeros((128, N), np.float32)
        pbc = np.zeros((128, 1), np.float32)
        for lbh, bh in enumerate(bhs):
            b, h = bh // H, bh % H
            r0 = lbh * 64
            qT[r0:r0 + 64, :] = q[b, h].T
            kTp[r0:r0 + 64, :] = k[b, h].T
            vTp[r0:r0 + 64, :] = v[b, h].T
            for i8, d in enumerate(ROT):
                for t in range(8):
                    r = _rrot(lbh, i8, t)
                    vsh[r, :] = _shift_np(
                        np.asarray(v[b, h, :, T_CH[t]]), d)
                for p in range(2):
                    for cs in range(2):
                        r = _rtc(s, lbh, i8, p, cs)
                        y128[r, :] = y_pre[b, h, :, p]
                        z128[r, :] = _shift_np(
                            np.asarray(z_pre[b, h, :, p]), d)
                        g128[r, 0] = phase_gain[i8, h, p]
                        b128[r, 0] = phase_base[i8, h, p] + (
                            np.pi / 2.0 if cs == 0 else 0.0)
            for i in range(J):
                pbc[_rsc(i, lbh), 0] = pb[i, h]
        m[f"qT{s}"] = _bf16(qT)
        m[f"kTp{s}"] = _bf16(kTp)
        m[f"vTp{s}"] = _bf16(vTp)
        m[f"vsh{s}"] = _bf16(vsh)
        m[f"pbc{s}"] = pbc
    m["y128"] = _bf16(y128)
    m["z128"] = _bf16(z128)
    m["g128"] = g128
    m["b128"] = b128
    return m


def make_in_maps(q, k, v, pb, se, phase_base, phase_gain, y_pre, z_pre):
    shared = _shared_consts(np.asarray(se))
    args = [np.asarray(x) for x in
            (q, k, v, pb, se, phase_base, phase_gain, y_pre, z_pre)]
    return [_core_inputs(c, *args, shared) for c in range(NCORES)]


def assemble_output(results):
    out = np.zeros((B, H, N, HD), np.float32)
    for core in range(NCORES):
        for s in range(2):
            outT = np.asarray(results[core][f"outT{s}"], dtype=np.float32)
            for lbh in range(2):
                bh = 4 * core + 2 * s + lbh
                b, h = bh // H, bh % H
                out[b, h] = outT[lbh * 64:(lbh + 1) * 64, :].T
    return out


def kernel(**inputs):
    from concourse.bass_utils import run_bass_kernel_spmd

    nc = get_program()
    in_maps = make_in_maps(**inputs)
    res = run_bass_kernel_spmd(nc, in_maps, core_ids=list(range(NCORES)))
    return assemble_output(res.results)


if __name__ == "__main__":
    get_program()
    print("program built + compiled OK")


# revision 50
# speedup vs baseline: 1.0374x; 1.0374x over previous
"""Trainium2 Bass kernel for DSQG attention (J=12 causal-offset sparse attention).

Sharding: data-parallel over (B,H): 32 bh-slices -> 8 cores x 4 bh.
Each core processes its 4 bh as 2 stacked pairs s in a transposed layout
[128 = 2bh x 64hd, N] so every sequence shift is a free-dim AP offset.

v3 (bf16 overhaul, 519us -> 221us cost-model):
  - all data bf16: PE matmuls 1 cyc/row (vs 4 for fp32), DVE TT 2x mode,
    half the HBM traffic. rel err 6.9e-3 (tol 2e-2).
  - unified score layout: score/e row = 32*a + 2*g + bh (i = 4g+a) so all
    12 offsets live in ONE [128, 512] PSUM tile -> 1 exp ACT op per half,
    single-matmul denominator / rot-broadcast / e-broadcast selectors.
  - score products k_sh*q split gpsimd/DVE (16/8 of 24 halves; walrus
    rejects STT on Pool, so +q*se_i rides a seG matmul into the same PSUM
    accumulation group).
  - [D]: PE broadcasts e_i (bsel), ACT copies PSUM->SBUF bf16, DVE does
    the e*v muls in 2x mode, and PE identity-matmuls accumulate the 12
    products in PSUM (replaces the 11-deep DVE add chain); the rotation
    correction (rotred) accumulates into the same PSUM group.
  - theta/sin computed once for BOTH stacks (128 compact rows = 2s x 2bh
    x 8i x 2p x 2cs) in a pre-pass: ACT loads the trig table once, exp
    table once (baseline reloaded tables 16x); cos-1 pre-folded into
    trig_full via a per-row scalar add; one range-wrap (|theta|<3pi).
  - DMA order tuned: chunk-0 q/k slices first, zero-padding memset
    on-chip instead of DMA'd, outputs bf16.
"""

import sys

for _p in ("/opt/trn_rl_repo", "/root/.axon_site/_ro/trn_rl_repo"):
    if _p not in sys.path:
        sys.path.insert(0, _p)

import numpy as np

OFFSETS = (1, 2, 4, 8, 16, 64, 96, 192, 384, 512, 768, 1024)
J = 12
B, H, N, HD = 2, 16, 4096, 64
PAD = 1024
NP_ = N + PAD
CH = 1024            # chunk width for SBUF/elementwise ops
CHA = 512            # PSUM-facing sub-chunk (one bank)
NCHUNK = N // CH
SC = 1.0 / 8.0
NCORES = 8
ROT = OFFSETS[4:]    # 8 rotating offsets (abs i = 4..11)
T_P = (0, 0, 0, 0, 1, 1, 1, 1)      # phase pair per term slot t
T_CH = (0, 1, 0, 1, 2, 3, 2, 3)     # v channel per t
T_CS = (0, 0, 1, 1, 0, 0, 1, 1)     # 0 = cos branch, 1 = sin branch

# score/e row for offset i = 4g + a (a = i%4, g = i//4), local bh in {0,1}
def _rsc(i, bh):
    a, g = i % 4, i // 4
    return 32 * a + 2 * g + bh

# rot-stack row (per s)
def _rrot(bh, i8, t):
    return 64 * bh + 8 * i8 + t

# trig compact row (shared over both s)
def _rtc(s, bh, i8, p, cs):
    return 64 * s + 32 * bh + 4 * i8 + 2 * p + cs

# engine assignment knobs
POOL_NPROD = 16                 # of 24 half-products per (c,s) on gpsimd
POOL_NPROD_RAMP = 2            # chunk-0 split (DVE idle during ramp)
ACT_COPY_D = set(range(12))     # offsets whose e-broadcast is ACT-copied to bf16
W_COPIES = ("trig8", "erps", "rotps", "rbps")   # extra PSUM->bf16 ACT copies

_PROGRAM = None


def _build_program():
    import concourse.tile as tile
    from concourse import bacc, mybir

    f32 = mybir.dt.float32
    bf16 = mybir.dt.bfloat16
    AluOp = mybir.AluOpType
    Act = mybir.ActivationFunctionType

    nc = bacc.Bacc()
    dp = nc.declare_dram_parameter

    ins = {}
    for s in range(2):
        ins[f"qT{s}"] = dp(f"qT{s}", [128, N], bf16, isOutput=False)
        ins[f"kTp{s}"] = dp(f"kTp{s}", [128, N], bf16, isOutput=False)
        ins[f"vTp{s}"] = dp(f"vTp{s}", [128, N], bf16, isOutput=False)
        ins[f"vsh{s}"] = dp(f"vsh{s}", [128, N], bf16, isOutput=False)
        ins[f"pbc{s}"] = dp(f"pbc{s}", [128, 1], f32, isOutput=False)
    ins["y128"] = dp("y128", [128, N], bf16, isOutput=False)
    ins["z128"] = dp("z128", [128, N], bf16, isOutput=False)
    ins["g128"] = dp("g128", [128, 1], f32, isOutput=False)
    ins["b128"] = dp("b128", [128, 1], f32, isOutput=False)
    ins["onesG"] = dp("onesG", [128, 3 * 32], bf16, isOutput=False)
    ins["esel"] = dp("esel", [128, 2], bf16, isOutput=False)
    ins["bsel"] = dp("bsel", [128, J * 128], bf16, isOutput=False)
    ins["rotsel"] = dp("rotsel", [128, 128], bf16, isOutput=False)
    ins["rotred"] = dp("rotred", [128, 128], bf16, isOutput=False)
    ins["rsel"] = dp("rsel", [64, 2 * 128], bf16, isOutput=False)
    ins["texp"] = dp("texp", [128, 2 * 128], bf16, isOutput=False)
    ins["cm1"] = dp("cm1", [128, 1], f32, isOutput=False)
    ins["ident"] = dp("ident", [128, 128], bf16, isOutput=False)
    ins["cmask"] = dp("cmask", [128, CH], bf16, isOutput=False)
    ins["seG"] = dp("seG", [128, 128], bf16, isOutput=False)
    outs = [dp(f"outT{s}", [128, N], bf16, isOutput=True) for s in range(2)]

    with tile.TileContext(nc) as tc:
        with (
            nc.allow_low_precision(
                reason="bf16 weighted-value accumulation; rel-tol 2e-2"),
            tc.tile_pool(name="consts", bufs=1) as cpool,
            tc.tile_pool(name="data", bufs=1) as dpool,
            tc.tile_pool(name="work", bufs=2) as wpool,
            tc.tile_pool(name="ecp", bufs=6) as ecpool,
            tc.tile_pool(name="prods", bufs=24) as ppool,
            tc.tile_pool(name="bcast", bufs=4) as bpool,
            tc.tile_pool(name="tmpp", bufs=3) as tpool,
            tc.tile_pool(name="mmp", bufs=12) as mmpool,
            tc.tile_pool(name="thp", bufs=1) as thpool,
            tc.tile_pool(name="psA", bufs=1, space="PSUM") as psA,   # scps
            tc.tile_pool(name="psB", bufs=1, space="PSUM") as psB,   # e-bcast
            tc.tile_pool(name="psACC", bufs=1, space="PSUM") as psACC,  # acc
            tc.tile_pool(name="psC", bufs=2, space="PSUM") as psC,   # rot seq
            tc.tile_pool(name="psD", bufs=1, space="PSUM") as psD,   # den/rb
        ):
            # DMA order tuned for ramp: chunk-0 score path first.
            qT = [None, None]
            kTp = [None, None]
            vTp = [None, None]
            vsh = [None, None]
            c_pbc = [None, None]
            qT[0] = dpool.tile([128, N], bf16, tag="qT0", name="qT0")
            nc.sync.dma_start(out=qT[0][:, 0:CH], in_=ins["qT0"][:, 0:CH])
            kTp[0] = dpool.tile([128, NP_], bf16, tag="kTp0", name="kTp0")
            nc.gpsimd.memset(kTp[0][:, 0:PAD], 0.0)
            nc.sync.dma_start(out=kTp[0][:, PAD:PAD + CH],
                              in_=ins["kTp0"][:, 0:CH])
            c_onesG = cpool.tile([128, 3 * 32], bf16, tag="c_onesG")
            nc.sync.dma_start(out=c_onesG, in_=ins["onesG"][:])
            c_seG = cpool.tile([128, 128], bf16, tag="c_seG")
            nc.sync.dma_start(out=c_seG, in_=ins["seG"][:])
            c_pbc[0] = cpool.tile([128, 1], f32, tag="c_pbc0", name="c_pbc0")
            nc.sync.dma_start(out=c_pbc[0], in_=ins["pbc0"][:])
            c_cmask = cpool.tile([128, CH], bf16, tag="c_cmask")
            nc.sync.dma_start(out=c_cmask, in_=ins["cmask"][:])
            nc.sync.dma_start(out=qT[0][:, CH:], in_=ins["qT0"][:, CH:])
            nc.sync.dma_start(out=kTp[0][:, PAD + CH:],
                              in_=ins["kTp0"][:, CH:])
            qT[1] = dpool.tile([128, N], bf16, tag="qT1", name="qT1")
            nc.sync.dma_start(out=qT[1], in_=ins["qT1"][:])
            kTp[1] = dpool.tile([128, NP_], bf16, tag="kTp1", name="kTp1")
            nc.gpsimd.memset(kTp[1][:, 0:PAD], 0.0)
            nc.sync.dma_start(out=kTp[1][:, PAD:], in_=ins["kTp1"][:])
            c_pbc[1] = cpool.tile([128, 1], f32, tag="c_pbc1", name="c_pbc1")
            nc.sync.dma_start(out=c_pbc[1], in_=ins["pbc1"][:])
            c_esel = cpool.tile([128, 2], bf16, tag="c_esel")
            nc.sync.dma_start(out=c_esel, in_=ins["esel"][:])
            c_bsel = cpool.tile([128, J * 128], bf16, tag="c_bsel")
            nc.sync.dma_start(out=c_bsel, in_=ins["bsel"][:])
            c_ident = cpool.tile([128, 128], bf16, tag="c_ident")
            nc.sync.dma_start(out=c_ident, in_=ins["ident"][:])
            vTp[0] = dpool.tile([128, NP_], bf16, tag="vTp0", name="vTp0")
            nc.gpsimd.memset(vTp[0][:, 0:PAD], 0.0)
            nc.sync.dma_start(out=vTp[0][:, PAD:], in_=ins["vTp0"][:])
            vTp[1] = dpool.tile([128, NP_], bf16, tag="vTp1", name="vTp1")
            nc.gpsimd.memset(vTp[1][:, 0:PAD], 0.0)
            nc.sync.dma_start(out=vTp[1][:, PAD:], in_=ins["vTp1"][:])
            y128 = dpool.tile([128, N], bf16, tag="y128")
            nc.sync.dma_start(out=y128, in_=ins["y128"][:])
            z128 = dpool.tile([128, N], bf16, tag="z128")
            nc.sync.dma_start(out=z128, in_=ins["z128"][:])
            c_g128 = cpool.tile([128, 1], f32, tag="c_g128")
            nc.sync.dma_start(out=c_g128, in_=ins["g128"][:])
            c_b128 = cpool.tile([128, 1], f32, tag="c_b128")
            nc.sync.dma_start(out=c_b128, in_=ins["b128"][:])
            vsh[0] = dpool.tile([128, N], bf16, tag="vsh0", name="vsh0")
            nc.sync.dma_start(out=vsh[0], in_=ins["vsh0"][:])
            c_rotsel = cpool.tile([128, 128], bf16, tag="c_rotsel")
            nc.sync.dma_start(out=c_rotsel, in_=ins["rotsel"][:])
            c_rotred = cpool.tile([128, 128], bf16, tag="c_rotred")
            nc.sync.dma_start(out=c_rotred, in_=ins["rotred"][:])
            c_rsel = cpool.tile([64, 2 * 128], bf16, tag="c_rsel")
            nc.sync.dma_start(out=c_rsel, in_=ins["rsel"][:])
            c_texp = cpool.tile([128, 2 * 128], bf16, tag="c_texp")
            nc.sync.dma_start(out=c_texp, in_=ins["texp"][:])
            c_cm1 = cpool.tile([128, 1], f32, tag="c_cm1")
            nc.sync.dma_start(out=c_cm1, in_=ins["cm1"][:])
            vsh[1] = dpool.tile([128, N], bf16, tag="vsh1", name="vsh1")
            nc.sync.dma_start(out=vsh[1], in_=ins["vsh1"][:])

            # ---------- trig pre-pass: trig_full for BOTH s ----------
            trig_full = dpool.tile([128, N], bf16, tag="trig_full")
            for pc in range(NCHUNK):
                n0 = pc * CH
                ub = wpool.tile([128, CH], bf16, tag="ub")
                nc.vector.tensor_mul(ub, y128[:, n0:n0 + CH],
                                     z128[:, n0:n0 + CH])
                th = thpool.tile([128, CH], f32, tag="th")
                nc.vector.tensor_scalar(
                    out=th, in0=ub,
                    scalar1=c_g128[:, 0:1], scalar2=c_b128[:, 0:1],
                    op0=AluOp.mult, op1=AluOp.add,
                )
                nc.vector.add_range_wrap(th, th, 0.0, np.pi, 2.0 * np.pi)
                trg = wpool.tile([128, CH], bf16, tag="trg")
                nc.scalar.activation(out=trg, in_=th,
                                     func=Act.Sin, bias=0.0, scale=1.0)
                nc.vector.tensor_scalar_add(trig_full[:, n0:n0 + CH],
                                            trg, c_cm1[:, 0:1])

            def phase_A(c):
                n0 = c * CH
                # ---------- [A] scores + exp (per s) ----------
                nprod = POOL_NPROD_RAMP if c == 0 else POOL_NPROD
                ec = [None, None]
                for s in range(2):
                    ec[s] = ecpool.tile([128, CH], bf16, tag="ec",
                                        name=f"ec{s}")
                    for hf in range(2):
                        h0 = hf * CHA
                        scps = psA.tile([128, CHA], f32, tag="scps")
                        for i, d in enumerate(OFFSETS):
                            a, g = i % 4, i // 4
                            eng = (nc.gpsimd if (2 * i + hf) % 24 < nprod
                                   else nc.vector)
                            pr = ppool.tile([128, CHA], bf16, tag="pr")
                            eng.tensor_mul(
                                pr,
                                kTp[s][:, PAD - d + n0 + h0:
                                       PAD - d + n0 + h0 + CHA],
                                qT[s][:, n0 + h0:n0 + h0 + CHA],
                            )
                            nc.tensor.matmul(
                                out=scps[32 * a:32 * a + 32, :],
                                lhsT=c_onesG[:, 32 * g:32 * g + 32],
                                rhs=pr,
                                start=(g == 0), stop=False,
                                tile_position=(0, 32 * a),
                            )
                        nc.tensor.matmul(
                            out=scps,
                            lhsT=c_seG,
                            rhs=qT[s][:, n0 + h0:n0 + h0 + CHA],
                            start=False, stop=True,
                        )
                        nc.scalar.activation(
                            out=ec[s][:, h0:h0 + CHA], in_=scps,
                            func=Act.Exp, bias=c_pbc[s][:, 0:1], scale=SC,
                        )
                    if c == 0:
                        nc.vector.tensor_mul(ec[s], ec[s], c_cmask)

                # ---------- denominators (both s share recip) ----------
                rcb = wpool.tile([34, CH], bf16, tag="rcb")
                for hf in range(2):
                    h0 = hf * CHA
                    denps = psD.tile([128, CHA], f32, tag="dn", name="denps")
                    for s in range(2):
                        nc.tensor.matmul(
                            out=denps[32 * s:32 * s + 2, :],
                            lhsT=c_esel,
                            rhs=ec[s][:, h0:h0 + CHA],
                            start=True, stop=True,
                        )
                    rf = tpool.tile([34, CHA], f32, tag="rf")
                    nc.vector.tensor_scalar_add(rf, denps[0:34, :], 1e-30)
                    nc.vector.reciprocal(rcb[:, h0:h0 + CHA], rf)
                return ec, rcb

            def phase_B(c, ec, rcb):
                n0 = c * CH
                for s in range(2):
                    accps = psACC.tile([128, CH], f32, tag="accps")
                    # ---------- [D] e-broadcast + weighted v ----------
                    mms = []
                    for i, d in enumerate(OFFSETS):
                        bps = psB.tile([128, CH], f32, tag="bps")
                        for hf in range(2):
                            h0 = hf * CHA
                            nc.tensor.matmul(
                                out=bps[:, h0:h0 + CHA],
                                lhsT=c_bsel[:, i * 128:i * 128 + 128],
                                rhs=ec[s][:, h0:h0 + CHA],
                                start=True, stop=True,
                            )
                        vsl = vTp[s][:, PAD - d + n0: PAD - d + n0 + CH]
                        mm = mmpool.tile([128, CH], bf16, tag="mm")
                        if i in ACT_COPY_D:
                            bcb = bpool.tile([128, CH], bf16, tag="bcb")
                            nc.scalar.activation(out=bcb, in_=bps,
                                                 func=Act.Copy, bias=0.0,
                                                 scale=1.0)
                            nc.vector.tensor_mul(mm, bcb, vsl)
                        else:
                            nc.vector.tensor_mul(mm, bps, vsl)
                        mms.append(mm)
                    for i, mm in enumerate(mms):
                        for hf in range(2):
                            h0 = hf * CHA
                            nc.tensor.matmul(
                                out=accps[:, h0:h0 + CHA],
                                lhsT=c_ident,
                                rhs=mm[:, h0:h0 + CHA],
                                start=(i == 0), stop=False,
                            )

                    # ---------- [R] rotation correction (into accps) ----
                    for hf in range(2):
                        h0 = hf * CHA
                        t8ps = psC.tile([128, CHA], f32, tag="rps",
                                        name="t8ps")
                        nc.tensor.matmul(
                            out=t8ps,
                            lhsT=c_texp[:, s * 128:s * 128 + 128],
                            rhs=trig_full[:, n0 + h0:n0 + h0 + CHA],
                            start=True, stop=True,
                        )
                        vss = vsh[s][:, n0 + h0:n0 + h0 + CHA]
                        prot = tpool.tile([128, CHA], bf16, tag="prot")
                        nc.vector.tensor_mul(prot, t8ps, vss)
                        erps = psC.tile([128, CHA], f32, tag="rps",
                                        name="erps")
                        nc.tensor.matmul(
                            out=erps,
                            lhsT=c_rotsel,
                            rhs=ec[s][:, h0:h0 + CHA],
                            start=True, stop=True,
                        )
                        vful = tpool.tile([128, CHA], bf16, tag="vful")
                        nc.vector.tensor_mul(vful, erps, prot)
                        nc.tensor.matmul(
                            out=accps[:, h0:h0 + CHA],
                            lhsT=c_rotred,
                            rhs=vful,
                            start=False, stop=True,
                        )

                    # ---------- [E] normalize + store ----------
                    outcb = wpool.tile([128, CH], bf16, tag="outcb")
                    for hf in range(2):
                        h0 = hf * CHA
                        rbps = psC.tile([128, CHA], f32, tag="rps",
                                        name="rbps")
                        nc.tensor.matmul(
                            out=rbps,
                            lhsT=c_rsel[32 * s:32 * s + 2, s * 128:s * 128 + 128],
                            rhs=rcb[32 * s:32 * s + 2, h0:h0 + CHA],
                            start=True, stop=True,
                        )
                        rbb = bpool.tile([128, CHA], bf16, tag="rbb")
                        nc.scalar.activation(out=rbb, in_=rbps,
                                             func=Act.Copy, bias=0.0,
                                             scale=1.0)
                        nc.vector.tensor_mul(outcb[:, h0:h0 + CHA],
                                             accps[:, h0:h0 + CHA], rbb)
                    nc.sync.dma_start(out=outs[s][:, n0:n0 + CH], in_=outcb)

            for c in range(NCHUNK):
                ec_c, rcb_c = phase_A(c)
                phase_B(c, ec_c, rcb_c)

    nc.compile()
    return nc


def get_program():
    global _PROGRAM
    if _PROGRAM is None:
        _PROGRAM = _build_program()
    return _PROGRAM


def _shift_np(x, d):
    """out[n] = x[n-d], zeros for n < d; shift along axis 0."""
    out = np.zeros_like(x)
    out[d:] = x[:-d] if d > 0 else x
    return out


def _bf16(x):
    import ml_dtypes
    return np.asarray(x, dtype=np.float32).astype(ml_dtypes.bfloat16)


def _shared_consts(se_global):
    c = {}
    onesG = np.zeros((128, 3 * 32), np.float32)
    for g in range(3):
        for bh in range(2):
            onesG[64 * bh:64 * bh + 64, 32 * g + 2 * g + bh] = 1.0
    c["onesG"] = _bf16(onesG)
    esel = np.zeros((128, 2), np.float32)
    for i in range(J):
        for bh in range(2):
            esel[_rsc(i, bh), bh] = 1.0
    c["esel"] = _bf16(esel)
    bsel = np.zeros((128, J * 128), np.float32)
    for i in range(J):
        for bh in range(2):
            for hd in range(64):
                bsel[_rsc(i, bh), i * 128 + 64 * bh + hd] = 1.0
    c["bsel"] = _bf16(bsel)
    rotsel = np.zeros((128, 128), np.float32)
    for bh in range(2):
        for i8 in range(8):
            abs_i = i8 + 4
            for t in range(8):
                rotsel[_rsc(abs_i, bh), _rrot(bh, i8, t)] = 1.0
    c["rotsel"] = _bf16(rotsel)
    rotred = np.zeros((128, 128), np.float32)
    sign_map = {0: ((0, 1.0), (3, -1.0)), 1: ((1, 1.0), (2, 1.0)),
                2: ((4, 1.0), (7, -1.0)), 3: ((5, 1.0), (6, 1.0))}
    for bh in range(2):
        for ch in range(4):
            col = bh * 64 + ch
            for i8 in range(8):
                for t, sgn in sign_map[ch]:
                    rotred[_rrot(bh, i8, t), col] = sgn
    c["rotred"] = _bf16(rotred)
    rsel = np.zeros((64, 2 * 128), np.float32)
    for s in range(2):
        for bh in range(2):
            rsel[32 * s + bh, s * 128 + 64 * bh: s * 128 + 64 * bh + 64] = 1.0
    c["rsel"] = _bf16(rsel)
    texp = np.zeros((128, 2 * 128), np.float32)
    for s in range(2):
        for bh in range(2):
            for i8 in range(8):
                for p in range(2):
                    for cs in range(2):
                        src = _rtc(s, bh, i8, p, cs)
                        for c2 in range(2):
                            t = 4 * p + 2 * cs + c2
                            texp[src, s * 128 + _rrot(bh, i8, t)] = 1.0
    c["texp"] = _bf16(texp)
    cm1 = np.zeros((128, 1), np.float32)
    for s in range(2):
        for bh in range(2):
            for i8 in range(8):
                for p in range(2):
                    cm1[_rtc(s, bh, i8, p, 0), 0] = -1.0
    c["cm1"] = cm1
    c["ident"] = _bf16(np.eye(128, dtype=np.float32))
    seG = np.zeros((128, 128), np.float32)
    for i in range(J):
        for bh in range(2):
            for hd in range(64):
                seG[64 * bh + hd, _rsc(i, bh)] = se_global[i, hd]
    c["seG"] = _bf16(seG)
    cmask = np.ones((128, CH), np.float32)
    for i, d in enumerate(OFFSETS):
        r = _rsc(i, 0)
        cmask[r:r + 2, 0:d] = 0.0
    c["cmask"] = _bf16(cmask)
    return c


def _core_inputs(core, q, k, v, pb, se, phase_base, phase_gain, y_pre, z_pre,
                 shared):
    m = dict(shared)
    y128 = np.zeros((128, N), np.float32)
    z128 = np.zeros((128, N), np.float32)
    g128 = np.zeros((128, 1), np.float32)
    b128 = np.zeros((128, 1), np.float32)
    for s in range(2):
        bhs = [4 * core + 2 * s, 4 * core + 2 * s + 1]
        qT = np.zeros((128, N), np.float32)
        kTp = np.zeros((128, N), np.float32)
        vTp = np.zeros((128, N), np.float32)
        vsh = np.zeros((128, N), np.float32)
        pbc = np.zeros((128, 1), np.float32)
        for lbh, bh in enumerate(bhs):
            b, h = bh // H, bh % H
            r0 = lbh * 64
            qT[r0:r0 + 64, :] = q[b, h].T
            kTp[r0:r0 + 64, :] = k[b, h].T
            vTp[r0:r0 + 64, :] = v[b, h].T
            for i8, d in enumerate(ROT):
                for t in range(8):
                    r = _rrot(lbh, i8, t)
                    vsh[r, :] = _shift_np(
                        np.asarray(v[b, h, :, T_CH[t]]), d)
                for p in range(2):
                    for cs in range(2):
                        r = _rtc(s, lbh, i8, p, cs)
                        y128[r, :] = y_pre[b, h, :, p]
                        z128[r, :] = _shift_np(
                            np.asarray(z_pre[b, h, :, p]), d)
                        g128[r, 0] = phase_gain[i8, h, p]
                        b128[r, 0] = phase_base[i8, h, p] + (
                            np.pi / 2.0 if cs == 0 else 0.0)
            for i in range(J):
                pbc[_rsc(i, lbh), 0] = pb[i, h]
        m[f"qT{s}"] = _bf16(qT)
        m[f"kTp{s}"] = _bf16(kTp)
        m[f"vTp{s}"] = _bf16(vTp)
        m[f"vsh{s}"] = _bf16(vsh)
        m[f"pbc{s}"] = pbc
    m["y128"] = _bf16(y128)
    m["z128"] = _bf16(z128)
    m["g128"] = g128
    m["b128"] = b128
    return m


def make_in_maps(q, k, v, pb, se, phase_base, phase_gain, y_pre, z_pre):
    shared = _shared_consts(np.asarray(se))
    args = [np.asarray(x) for x in
            (q, k, v, pb, se, phase_base, phase_gain, y_pre, z_pre)]
    return [_core_inputs(c, *args, shared) for c in range(NCORES)]


def assemble_output(results):
    out = np.zeros((B, H, N, HD), np.float32)
    for core in range(NCORES):
        for s in range(2):
            outT = np.asarray(results[core][f"outT{s}"], dtype=np.float32)
            for lbh in range(2):
                bh = 4 * core + 2 * s + lbh
                b, h = bh // H, bh % H
                out[b, h] = outT[lbh * 64:(lbh + 1) * 64, :].T
    return out


def kernel(**inputs):
    from concourse.bass_utils import run_bass_kernel_spmd

    nc = get_program()
    in_maps = make_in_maps(**inputs)
    res = run_bass_kernel_spmd(nc, in_maps, core_ids=list(range(NCORES)))
    return assemble_output(res.results)


if __name__ == "__main__":
    get_program()
    print("program built + compiled OK")


# revision 57
# speedup vs baseline: 1.0394x; 1.0020x over previous
"""Trainium2 Bass kernel for DSQG attention (J=12 causal-offset sparse attention).

Sharding: data-parallel over (B,H): 32 bh-slices -> 8 cores x 4 bh.
Each core processes its 4 bh as 2 stacked pairs s in a transposed layout
[128 = 2bh x 64hd, N] so every sequence shift is a free-dim AP offset.

v3 (bf16 overhaul, 519us -> 221us cost-model):
  - all data bf16: PE matmuls 1 cyc/row (vs 4 for fp32), DVE TT 2x mode,
    half the HBM traffic. rel err 6.9e-3 (tol 2e-2).
  - unified score layout: score/e row = 32*a + 2*g + bh (i = 4g+a) so all
    12 offsets live in ONE [128, 512] PSUM tile -> 1 exp ACT op per half,
    single-matmul denominator / rot-broadcast / e-broadcast selectors.
  - score products k_sh*q split gpsimd/DVE (16/8 of 24 halves; walrus
    rejects STT on Pool, so +q*se_i rides a seG matmul into the same PSUM
    accumulation group).
  - [D]: PE broadcasts e_i (bsel), ACT copies PSUM->SBUF bf16, DVE does
    the e*v muls in 2x mode, and PE identity-matmuls accumulate the 12
    products in PSUM (replaces the 11-deep DVE add chain); the rotation
    correction (rotred) accumulates into the same PSUM group.
  - theta/sin computed once for BOTH stacks (128 compact rows = 2s x 2bh
    x 8i x 2p x 2cs) in a pre-pass: ACT loads the trig table once, exp
    table once (baseline reloaded tables 16x); cos-1 pre-folded into
    trig_full via a per-row scalar add; one range-wrap (|theta|<3pi).
  - DMA order tuned: chunk-0 q/k slices first, zero-padding memset
    on-chip instead of DMA'd, outputs bf16.
"""

import sys

for _p in ("/opt/trn_rl_repo", "/root/.axon_site/_ro/trn_rl_repo"):
    if _p not in sys.path:
        sys.path.insert(0, _p)

import numpy as np

OFFSETS = (1, 2, 4, 8, 16, 64, 96, 192, 384, 512, 768, 1024)
J = 12
B, H, N, HD = 2, 16, 4096, 64
PAD = 1024
NP_ = N + PAD
CH = 1024            # chunk width for SBUF/elementwise ops
CHA = 512            # PSUM-facing sub-chunk (one bank)
NCHUNK = N // CH
SC = 1.0 / 8.0
NCORES = 8
ROT = OFFSETS[4:]    # 8 rotating offsets (abs i = 4..11)
T_P = (0, 0, 0, 0, 1, 1, 1, 1)      # phase pair per term slot t
T_CH = (0, 1, 0, 1, 2, 3, 2, 3)     # v channel per t
T_CS = (0, 0, 1, 1, 0, 0, 1, 1)     # 0 = cos branch, 1 = sin branch

# score/e row for offset i = 4g + a (a = i%4, g = i//4), local bh in {0,1}
def _rsc(i, bh):
    a, g = i % 4, i // 4
    return 32 * a + 2 * g + bh

# rot-stack row (per s)
def _rrot(bh, i8, t):
    return 64 * bh + 8 * i8 + t

# trig compact row (shared over both s)
def _rtc(s, bh, i8, p, cs):
    return 64 * s + 32 * bh + 4 * i8 + 2 * p + cs

# engine assignment knobs
POOL_NPROD = 16                 # of 24 half-products per (c,s) on gpsimd
POOL_NPROD_RAMP = 2            # chunk-0 split (DVE idle during ramp)
ACT_COPY_D = set(range(12))     # offsets whose e-broadcast is ACT-copied to bf16
W_COPIES = ("trig8", "erps", "rotps", "rbps")   # extra PSUM->bf16 ACT copies

_PROGRAM = None


def _build_program():
    import concourse.tile as tile
    from concourse import bacc, mybir

    f32 = mybir.dt.float32
    bf16 = mybir.dt.bfloat16
    AluOp = mybir.AluOpType
    Act = mybir.ActivationFunctionType

    nc = bacc.Bacc()
    dp = nc.declare_dram_parameter

    ins = {}
    for s in range(2):
        ins[f"qT{s}"] = dp(f"qT{s}", [128, N], bf16, isOutput=False)
        ins[f"kTp{s}"] = dp(f"kTp{s}", [128, N], bf16, isOutput=False)
        ins[f"vTp{s}"] = dp(f"vTp{s}", [128, N], bf16, isOutput=False)
        ins[f"vsh{s}"] = dp(f"vsh{s}", [128, N], bf16, isOutput=False)
        ins[f"pbc{s}"] = dp(f"pbc{s}", [128, 1], f32, isOutput=False)
    ins["y128"] = dp("y128", [128, N], bf16, isOutput=False)
    ins["z128"] = dp("z128", [128, N], bf16, isOutput=False)
    ins["g128"] = dp("g128", [128, 1], f32, isOutput=False)
    ins["b128"] = dp("b128", [128, 1], f32, isOutput=False)
    ins["onesG"] = dp("onesG", [128, 3 * 32], bf16, isOutput=False)
    ins["esel"] = dp("esel", [128, 2], bf16, isOutput=False)
    ins["bsel"] = dp("bsel", [128, J * 128], bf16, isOutput=False)
    ins["rotsel"] = dp("rotsel", [128, 128], bf16, isOutput=False)
    ins["rotred"] = dp("rotred", [128, 128], bf16, isOutput=False)
    ins["rsel"] = dp("rsel", [64, 2 * 128], bf16, isOutput=False)
    ins["texp"] = dp("texp", [128, 2 * 128], bf16, isOutput=False)
    ins["cm1"] = dp("cm1", [128, 1], f32, isOutput=False)
    ins["ident"] = dp("ident", [128, 128], bf16, isOutput=False)
    ins["cmask"] = dp("cmask", [128, CH], bf16, isOutput=False)
    ins["seG"] = dp("seG", [128, 128], bf16, isOutput=False)
    outs = [dp(f"outT{s}", [128, N], bf16, isOutput=True) for s in range(2)]

    with tile.TileContext(nc) as tc:
        with (
            nc.allow_low_precision(
                reason="bf16 weighted-value accumulation; rel-tol 2e-2"),
            tc.tile_pool(name="consts", bufs=1) as cpool,
            tc.tile_pool(name="data", bufs=1) as dpool,
            tc.tile_pool(name="work", bufs=2) as wpool,
            tc.tile_pool(name="ecp", bufs=6) as ecpool,
            tc.tile_pool(name="prods", bufs=24) as ppool,
            tc.tile_pool(name="bcast", bufs=4) as bpool,
            tc.tile_pool(name="tmpp", bufs=3) as tpool,
            tc.tile_pool(name="mmp", bufs=12) as mmpool,
            tc.tile_pool(name="thp", bufs=1) as thpool,
            tc.tile_pool(name="psA", bufs=1, space="PSUM") as psA,   # scps
            tc.tile_pool(name="psB", bufs=1, space="PSUM") as psB,   # e-bcast
            tc.tile_pool(name="psACC", bufs=2, space="PSUM") as psACC,  # acc
            tc.tile_pool(name="psC", bufs=2, space="PSUM") as psC,   # rot seq
            tc.tile_pool(name="psD", bufs=1, space="PSUM") as psD,   # den/rb
        ):
            # DMA order tuned for ramp: chunk-0 score path first.
            qT = [None, None]
            kTp = [None, None]
            vTp = [None, None]
            vsh = [None, None]
            c_pbc = [None, None]
            qT[0] = dpool.tile([128, N], bf16, tag="qT0", name="qT0")
            nc.sync.dma_start(out=qT[0][:, 0:CH], in_=ins["qT0"][:, 0:CH])
            kTp[0] = dpool.tile([128, NP_], bf16, tag="kTp0", name="kTp0")
            nc.gpsimd.memset(kTp[0][:, 0:PAD], 0.0)
            nc.sync.dma_start(out=kTp[0][:, PAD:PAD + CH],
                              in_=ins["kTp0"][:, 0:CH])
            c_onesG = cpool.tile([128, 3 * 32], bf16, tag="c_onesG")
            nc.sync.dma_start(out=c_onesG, in_=ins["onesG"][:])
            c_seG = cpool.tile([128, 128], bf16, tag="c_seG")
            nc.sync.dma_start(out=c_seG, in_=ins["seG"][:])
            c_pbc[0] = cpool.tile([128, 1], f32, tag="c_pbc0", name="c_pbc0")
            nc.sync.dma_start(out=c_pbc[0], in_=ins["pbc0"][:])
            c_cmask = cpool.tile([128, CH], bf16, tag="c_cmask")
            nc.sync.dma_start(out=c_cmask, in_=ins["cmask"][:])
            nc.sync.dma_start(out=qT[0][:, CH:], in_=ins["qT0"][:, CH:])
            nc.sync.dma_start(out=kTp[0][:, PAD + CH:],
                              in_=ins["kTp0"][:, CH:])
            qT[1] = dpool.tile([128, N], bf16, tag="qT1", name="qT1")
            nc.sync.dma_start(out=qT[1], in_=ins["qT1"][:])
            kTp[1] = dpool.tile([128, NP_], bf16, tag="kTp1", name="kTp1")
            nc.gpsimd.memset(kTp[1][:, 0:PAD], 0.0)
            nc.sync.dma_start(out=kTp[1][:, PAD:], in_=ins["kTp1"][:])
            c_pbc[1] = cpool.tile([128, 1], f32, tag="c_pbc1", name="c_pbc1")
            nc.sync.dma_start(out=c_pbc[1], in_=ins["pbc1"][:])
            c_esel = cpool.tile([128, 2], bf16, tag="c_esel")
            nc.sync.dma_start(out=c_esel, in_=ins["esel"][:])
            c_bsel = cpool.tile([128, J * 128], bf16, tag="c_bsel")
            nc.sync.dma_start(out=c_bsel, in_=ins["bsel"][:])
            c_ident = cpool.tile([128, 128], bf16, tag="c_ident")
            nc.sync.dma_start(out=c_ident, in_=ins["ident"][:])
            vTp[0] = dpool.tile([128, NP_], bf16, tag="vTp0", name="vTp0")
            nc.gpsimd.memset(vTp[0][:, 0:PAD], 0.0)
            nc.sync.dma_start(out=vTp[0][:, PAD:], in_=ins["vTp0"][:])
            vTp[1] = dpool.tile([128, NP_], bf16, tag="vTp1", name="vTp1")
            nc.gpsimd.memset(vTp[1][:, 0:PAD], 0.0)
            nc.sync.dma_start(out=vTp[1][:, PAD:], in_=ins["vTp1"][:])
            y128 = dpool.tile([128, N], bf16, tag="y128")
            nc.sync.dma_start(out=y128, in_=ins["y128"][:])
            z128 = dpool.tile([128, N], bf16, tag="z128")
            nc.sync.dma_start(out=z128, in_=ins["z128"][:])
            c_g128 = cpool.tile([128, 1], f32, tag="c_g128")
            nc.sync.dma_start(out=c_g128, in_=ins["g128"][:])
            c_b128 = cpool.tile([128, 1], f32, tag="c_b128")
            nc.sync.dma_start(out=c_b128, in_=ins["b128"][:])
            vsh[0] = dpool.tile([128, N], bf16, tag="vsh0", name="vsh0")
            nc.sync.dma_start(out=vsh[0], in_=ins["vsh0"][:])
            c_rotsel = cpool.tile([128, 128], bf16, tag="c_rotsel")
            nc.sync.dma_start(out=c_rotsel, in_=ins["rotsel"][:])
            c_rotred = cpool.tile([128, 128], bf16, tag="c_rotred")
            nc.sync.dma_start(out=c_rotred, in_=ins["rotred"][:])
            c_rsel = cpool.tile([64, 2 * 128], bf16, tag="c_rsel")
            nc.sync.dma_start(out=c_rsel, in_=ins["rsel"][:])
            c_texp = cpool.tile([128, 2 * 128], bf16, tag="c_texp")
            nc.sync.dma_start(out=c_texp, in_=ins["texp"][:])
            c_cm1 = cpool.tile([128, 1], f32, tag="c_cm1")
            nc.sync.dma_start(out=c_cm1, in_=ins["cm1"][:])
            vsh[1] = dpool.tile([128, N], bf16, tag="vsh1", name="vsh1")
            nc.sync.dma_start(out=vsh[1], in_=ins["vsh1"][:])

            # ---------- trig pre-pass: trig_full for BOTH s ----------
            trig_full = dpool.tile([128, N], bf16, tag="trig_full")
            for pc in range(NCHUNK):
                n0 = pc * CH
                ub = wpool.tile([128, CH], bf16, tag="ub")
                nc.vector.tensor_mul(ub, y128[:, n0:n0 + CH],
                                     z128[:, n0:n0 + CH])
                th = thpool.tile([128, CH], f32, tag="th")
                nc.vector.tensor_scalar(
                    out=th, in0=ub,
                    scalar1=c_g128[:, 0:1], scalar2=c_b128[:, 0:1],
                    op0=AluOp.mult, op1=AluOp.add,
                )
                nc.vector.add_range_wrap(th, th, 0.0, np.pi, 2.0 * np.pi)
                trg = wpool.tile([128, CH], bf16, tag="trg")
                nc.scalar.activation(out=trg, in_=th,
                                     func=Act.Sin, bias=0.0, scale=1.0)
                nc.vector.tensor_scalar_add(trig_full[:, n0:n0 + CH],
                                            trg, c_cm1[:, 0:1])

            def phase_A(c):
                n0 = c * CH
                # ---------- [A] scores + exp (per s) ----------
                nprod = POOL_NPROD_RAMP if c == 0 else POOL_NPROD
                ec = [None, None]
                for s in range(2):
                    ec[s] = ecpool.tile([128, CH], bf16, tag="ec",
                                        name=f"ec{s}")
                    for hf in range(2):
                        h0 = hf * CHA
                        scps = psA.tile([128, CHA], f32, tag="scps")
                        for i, d in enumerate(OFFSETS):
                            a, g = i % 4, i // 4
                            eng = (nc.gpsimd if (2 * i + hf) % 24 < nprod
                                   else nc.vector)
                            pr = ppool.tile([128, CHA], bf16, tag="pr")
                            eng.tensor_mul(
                                pr,
                                kTp[s][:, PAD - d + n0 + h0:
                                       PAD - d + n0 + h0 + CHA],
                                qT[s][:, n0 + h0:n0 + h0 + CHA],
                            )
                            nc.tensor.matmul(
                                out=scps[32 * a:32 * a + 32, :],
                                lhsT=c_onesG[:, 32 * g:32 * g + 32],
                                rhs=pr,
                                start=(g == 0), stop=False,
                                tile_position=(0, 32 * a),
                            )
                        nc.tensor.matmul(
                            out=scps,
                            lhsT=c_seG,
                            rhs=qT[s][:, n0 + h0:n0 + h0 + CHA],
                            start=False, stop=True,
                        )
                        nc.scalar.activation(
                            out=ec[s][:, h0:h0 + CHA], in_=scps,
                            func=Act.Exp, bias=c_pbc[s][:, 0:1], scale=SC,
                        )
                    if c == 0:
                        nc.vector.tensor_mul(ec[s], ec[s], c_cmask)

                # ---------- denominators (both s share recip) ----------
                rcb = wpool.tile([34, CH], bf16, tag="rcb")
                for hf in range(2):
                    h0 = hf * CHA
                    denps = psD.tile([128, CHA], f32, tag="dn", name="denps")
                    for s in range(2):
                        nc.tensor.matmul(
                            out=denps[32 * s:32 * s + 2, :],
                            lhsT=c_esel,
                            rhs=ec[s][:, h0:h0 + CHA],
                            start=True, stop=True,
                        )
                    rf = tpool.tile([34, CHA], f32, tag="rf")
                    nc.vector.tensor_scalar_add(rf, denps[0:34, :], 1e-30)
                    nc.vector.reciprocal(rcb[:, h0:h0 + CHA], rf)
                return ec, rcb

            def phase_B(c, ec, rcb):
                n0 = c * CH
                for s in range(2):
                    # ---------- [D] e-broadcast + weighted v ----------
                    mms = []
                    for i, d in enumerate(OFFSETS):
                        bps = psB.tile([128, CH], f32, tag="bps")
                        for hf in range(2):
                            h0 = hf * CHA
                            nc.tensor.matmul(
                                out=bps[:, h0:h0 + CHA],
                                lhsT=c_bsel[:, i * 128:i * 128 + 128],
                                rhs=ec[s][:, h0:h0 + CHA],
                                start=True, stop=True,
                            )
                        vsl = vTp[s][:, PAD - d + n0: PAD - d + n0 + CH]
                        mm = mmpool.tile([128, CH], bf16, tag="mm")
                        if i in ACT_COPY_D:
                            bcb = bpool.tile([128, CH], bf16, tag="bcb")
                            nc.scalar.activation(out=bcb, in_=bps,
                                                 func=Act.Copy, bias=0.0,
                                                 scale=1.0)
                            nc.vector.tensor_mul(mm, bcb, vsl)
                        else:
                            nc.vector.tensor_mul(mm, bps, vsl)
                        mms.append(mm)

                    outcb = wpool.tile([128, CH], bf16, tag="outcb")
                    for hf in range(2):
                        h0 = hf * CHA
                        accps = psACC.tile([128, CHA], f32, tag="accps")
                        for i, mm in enumerate(mms):
                            nc.tensor.matmul(
                                out=accps,
                                lhsT=c_ident,
                                rhs=mm[:, h0:h0 + CHA],
                                start=(i == 0), stop=False,
                            )
                        # ---------- [R] rotation correction (into accps) --
                        t8ps = psC.tile([128, CHA], f32, tag="rps",
                                        name="t8ps")
                        nc.tensor.matmul(
                            out=t8ps,
                            lhsT=c_texp[:, s * 128:s * 128 + 128],
                            rhs=trig_full[:, n0 + h0:n0 + h0 + CHA],
                            start=True, stop=True,
                        )
                        vss = vsh[s][:, n0 + h0:n0 + h0 + CHA]
                        prot = tpool.tile([128, CHA], bf16, tag="prot")
                        nc.vector.tensor_mul(prot, t8ps, vss)
                        erps = psC.tile([128, CHA], f32, tag="rps",
                                        name="erps")
                        nc.tensor.matmul(
                            out=erps,
                            lhsT=c_rotsel,
                            rhs=ec[s][:, h0:h0 + CHA],
                            start=True, stop=True,
                        )
                        vful = tpool.tile([128, CHA], bf16, tag="vful")
                        nc.vector.tensor_mul(vful, erps, prot)
                        nc.tensor.matmul(
                            out=accps,
                            lhsT=c_rotred,
                            rhs=vful,
                            start=False, stop=True,
                        )
                        # ---------- [E] normalize ----------
                        rbps = psC.tile([128, CHA], f32, tag="rps",
                                        name="rbps")
                        nc.tensor.matmul(
                            out=rbps,
                            lhsT=c_rsel[32 * s:32 * s + 2,
                                        s * 128:s * 128 + 128],
                            rhs=rcb[32 * s:32 * s + 2, h0:h0 + CHA],
                            start=True, stop=True,
                        )
                        rbb = bpool.tile([128, CHA], bf16, tag="rbb")
                        nc.scalar.activation(out=rbb, in_=rbps,
                                             func=Act.Copy, bias=0.0,
                                             scale=1.0)
                        nc.vector.tensor_mul(outcb[:, h0:h0 + CHA],
                                             accps, rbb)
                    nc.sync.dma_start(out=outs[s][:, n0:n0 + CH], in_=outcb)

            for c in range(NCHUNK):
                ec_c, rcb_c = phase_A(c)
                phase_B(c, ec_c, rcb_c)

    nc.compile()
    return nc


def get_program():
    global _PROGRAM
    if _PROGRAM is None:
        _PROGRAM = _build_program()
    return _PROGRAM


def _shift_np(x, d):
    """out[n] = x[n-d], zeros for n < d; shift along axis 0."""
    out = np.zeros_like(x)
    out[d:] = x[:-d] if d > 0 else x
    return out


def _bf16(x):
    import ml_dtypes
    return np.asarray(x, dtype=np.float32).astype(ml_dtypes.bfloat16)


def _shared_consts(se_global):
    c = {}
    onesG = np.zeros((128, 3 * 32), np.float32)
    for g in range(3):
        for bh in range(2):
            onesG[64 * bh:64 * bh + 64, 32 * g + 2 * g + bh] = 1.0
    c["onesG"] = _bf16(onesG)
    esel = np.zeros((128, 2), np.float32)
    for i in range(J):
        for bh in range(2):
            esel[_rsc(i, bh), bh] = 1.0
    c["esel"] = _bf16(esel)
    bsel = np.zeros((128, J * 128), np.float32)
    for i in range(J):
        for bh in range(2):
            for hd in range(64):
                bsel[_rsc(i, bh), i * 128 + 64 * bh + hd] = 1.0
    c["bsel"] = _bf16(bsel)
    rotsel = np.zeros((128, 128), np.float32)
    for bh in range(2):
        for i8 in range(8):
            abs_i = i8 + 4
            for t in range(8):
                rotsel[_rsc(abs_i, bh), _rrot(bh, i8, t)] = 1.0
    c["rotsel"] = _bf16(rotsel)
    rotred = np.zeros((128, 128), np.float32)
    sign_map = {0: ((0, 1.0), (3, -1.0)), 1: ((1, 1.0), (2, 1.0)),
                2: ((4, 1.0), (7, -1.0)), 3: ((5, 1.0), (6, 1.0))}
    for bh in range(2):
        for ch in range(4):
            col = bh * 64 + ch
            for i8 in range(8):
                for t, sgn in sign_map[ch]:
                    rotred[_rrot(bh, i8, t), col] = sgn
    c["rotred"] = _bf16(rotred)
    rsel = np.zeros((64, 2 * 128), np.float32)
    for s in range(2):
        for bh in range(2):
            rsel[32 * s + bh, s * 128 + 64 * bh: s * 128 + 64 * bh + 64] = 1.0
    c["rsel"] = _bf16(rsel)
    texp = np.zeros((128, 2 * 128), np.float32)
    for s in range(2):
        for bh in range(2):
            for i8 in range(8):
                for p in range(2):
                    for cs in range(2):
                        src = _rtc(s, bh, i8, p, cs)
                        for c2 in range(2):
                            t = 4 * p + 2 * cs + c2
                            texp[src, s * 128 + _rrot(bh, i8, t)] = 1.0
    c["texp"] = _bf16(texp)
    cm1 = np.zeros((128, 1), np.float32)
    for s in range(2):
        for bh in range(2):
            for i8 in range(8):
                for p in range(2):
                    cm1[_rtc(s, bh, i8, p, 0), 0] = -1.0
    c["cm1"] = cm1
    c["ident"] = _bf16(np.eye(128, dtype=np.float32))
    seG = np.zeros((128, 128), np.float32)
    for i in range(J):
        for bh in range(2):
            for hd in range(64):
                seG[64 * bh + hd, _rsc(i, bh)] = se_global[i, hd]
    c["seG"] = _bf16(seG)
    cmask = np.ones((128, CH), np.float32)
    for i, d in enumerate(OFFSETS):
        r = _rsc(i, 0)
        cmask[r:r + 2, 0:d] = 0.0
    c["cmask"] = _bf16(cmask)
    return c


def _core_inputs(core, q, k, v, pb, se, phase_base, phase_gain, y_pre, z_pre,
                 shared):
    m = dict(shared)
    y128 = np.zeros((128, N), np.float32)
    z128 = np.zeros((128, N), np.float32)
    g128 = np.zeros((128, 1), np.float32)
    b128 = np.zeros((128, 1), np.float32)
    for s in range(2):
        bhs = [4 * core + 2 * s, 4 * core + 2 * s + 1]
        qT = np.zeros((128, N), np.float32)
        kTp = np.zeros((128, N), np.float32)
        vTp = np.zeros((128, N), np.float32)
        vsh = np.zeros((128, N), np.float32)
        pbc = np.zeros((128, 1), np.float32)
        for lbh, bh in enumerate(bhs):
            b, h = bh // H, bh % H
            r0 = lbh * 64
            qT[r0:r0 + 64, :] = q[b, h].T
            kTp[r0:r0 + 64, :] = k[b, h].T
            vTp[r0:r0 + 64, :] = v[b, h].T
            for i8, d in enumerate(ROT):
                for t in range(8):
                    r = _rrot(lbh, i8, t)
                    vsh[r, :] = _shift_np(
                        np.asarray(v[b, h, :, T_CH[t]]), d)
                for p in range(2):
                    for cs in range(2):
                        r = _rtc(s, lbh, i8, p, cs)
                        y128[r, :] = y_pre[b, h, :, p]
                        z128[r, :] = _shift_np(
                            np.asarray(z_pre[b, h, :, p]), d)
                        g128[r, 0] = phase_gain[i8, h, p]
                        b128[r, 0] = phase_base[i8, h, p] + (
                            np.pi / 2.0 if cs == 0 else 0.0)
            for i in range(J):
                pbc[_rsc(i, lbh), 0] = pb[i, h]
        m[f"qT{s}"] = _bf16(qT)
        m[f"kTp{s}"] = _bf16(kTp)
        m[f"vTp{s}"] = _bf16(vTp)
        m[f"vsh{s}"] = _bf16(vsh)
        m[f"pbc{s}"] = pbc
    m["y128"] = _bf16(y128)
    m["z128"] = _bf16(z128)
    m["g128"] = g128
    m["b128"] = b128
    return m


def make_in_maps(q, k, v, pb, se, phase_base, phase_gain, y_pre, z_pre):
    shared = _shared_consts(np.asarray(se))
    args = [np.asarray(x) for x in
            (q, k, v, pb, se, phase_base, phase_gain, y_pre, z_pre)]
    return [_core_inputs(c, *args, shared) for c in range(NCORES)]


def assemble_output(results):
    out = np.zeros((B, H, N, HD), np.float32)
    for core in range(NCORES):
        for s in range(2):
            outT = np.asarray(results[core][f"outT{s}"], dtype=np.float32)
            for lbh in range(2):
                bh = 4 * core + 2 * s + lbh
                b, h = bh // H, bh % H
                out[b, h] = outT[lbh * 64:(lbh + 1) * 64, :].T
    return out


def kernel(**inputs):
    from concourse.bass_utils import run_bass_kernel_spmd

    nc = get_program()
    in_maps = make_in_maps(**inputs)
    res = run_bass_kernel_spmd(nc, in_maps, core_ids=list(range(NCORES)))
    return assemble_output(res.results)


if __name__ == "__main__":
    get_program()
    print("program built + compiled OK")


# revision 67
# speedup vs baseline: 1.2346x; 1.1878x over previous
"""Trainium2 Bass kernel for DSQG attention (J=12 causal-offset sparse attention).

Sharding: data-parallel over (B,H): 32 bh-slices -> 8 cores x 4 bh.
Each core processes its 4 bh as 2 stacked pairs s in a transposed layout
[128 = 2bh x 64hd, N] so every sequence shift is a free-dim AP offset.

v3 (bf16 overhaul, 519us -> 221us cost-model):
  - all data bf16: PE matmuls 1 cyc/row (vs 4 for fp32), DVE TT 2x mode,
    half the HBM traffic. rel err 6.9e-3 (tol 2e-2).
  - unified score layout: score/e row = 32*a + 2*g + bh (i = 4g+a) so all
    12 offsets live in ONE [128, 512] PSUM tile -> 1 exp ACT op per half,
    single-matmul denominator / rot-broadcast / e-broadcast selectors.
  - score products k_sh*q split gpsimd/DVE (16/8 of 24 halves; walrus
    rejects STT on Pool, so +q*se_i rides a seG matmul into the same PSUM
    accumulation group).
  - [D]: PE broadcasts e_i (bsel), ACT copies PSUM->SBUF bf16, DVE does
    the e*v muls in 2x mode, and PE identity-matmuls accumulate the 12
    products in PSUM (replaces the 11-deep DVE add chain); the rotation
    correction (rotred) accumulates into the same PSUM group.
  - theta/sin computed once for BOTH stacks (128 compact rows = 2s x 2bh
    x 8i x 2p x 2cs) in a pre-pass: ACT loads the trig table once, exp
    table once (baseline reloaded tables 16x); cos-1 pre-folded into
    trig_full via a per-row scalar add; one range-wrap (|theta|<3pi).
  - DMA order tuned: chunk-0 q/k slices first, zero-padding memset
    on-chip instead of DMA'd, outputs bf16.
"""

import sys

for _p in ("/opt/trn_rl_repo", "/root/.axon_site/_ro/trn_rl_repo"):
    if _p not in sys.path:
        sys.path.insert(0, _p)

import numpy as np

OFFSETS = (1, 2, 4, 8, 16, 64, 96, 192, 384, 512, 768, 1024)
J = 12
B, H, N, HD = 2, 16, 4096, 64
PAD = 1024
NP_ = N + PAD
CH = 1024            # chunk width for SBUF/elementwise ops
CHA = 512            # PSUM-facing sub-chunk (one bank)
NCHUNK = N // CH
SC = 1.0 / 8.0
NCORES = 8
ROT = OFFSETS[4:]    # 8 rotating offsets (abs i = 4..11)
T_P = (0, 0, 0, 0, 1, 1, 1, 1)      # phase pair per term slot t
T_CH = (0, 1, 0, 1, 2, 3, 2, 3)     # v channel per t
T_CS = (0, 0, 1, 1, 0, 0, 1, 1)     # 0 = cos branch, 1 = sin branch

# score/e row for offset i = 4g + a (a = i%4, g = i//4), local bh in {0,1}
def _rsc(i, bh):
    a, g = i % 4, i // 4
    return 32 * a + 2 * g + bh

# rot-stack row (per s)
def _rrot(bh, i8, t):
    return 64 * bh + 8 * i8 + t

# trig compact row (shared over both s)
def _rtc(s, bh, i8, p, cs):
    return 64 * s + 32 * bh + 4 * i8 + 2 * p + cs

# engine assignment knobs
POOL_NPROD = 16                 # of 24 half-products per (c,s) on gpsimd
POOL_NPROD_RAMP = 2            # chunk-0 split (DVE idle during ramp)
ACT_COPY_D = set(range(12))     # offsets whose e-broadcast is ACT-copied to bf16
W_COPIES = ("trig8", "erps", "rotps", "rbps")   # extra PSUM->bf16 ACT copies

_PROGRAM = None


def _build_program():
    import concourse.tile as tile
    from concourse import bacc, mybir

    f32 = mybir.dt.float32
    bf16 = mybir.dt.bfloat16
    AluOp = mybir.AluOpType
    Act = mybir.ActivationFunctionType

    nc = bacc.Bacc()
    dp = nc.declare_dram_parameter

    ins = {}
    for s in range(2):
        ins[f"qT{s}"] = dp(f"qT{s}", [128, N], bf16, isOutput=False)
        ins[f"kTp{s}"] = dp(f"kTp{s}", [128, N], bf16, isOutput=False)
        ins[f"vTp{s}"] = dp(f"vTp{s}", [128, N], bf16, isOutput=False)
        ins[f"vsh{s}"] = dp(f"vsh{s}", [128, N], bf16, isOutput=False)
        ins[f"pbc{s}"] = dp(f"pbc{s}", [128, 1], f32, isOutput=False)
    ins["y128"] = dp("y128", [128, N], bf16, isOutput=False)
    ins["z128"] = dp("z128", [128, N], bf16, isOutput=False)
    ins["g128"] = dp("g128", [128, 1], f32, isOutput=False)
    ins["b128"] = dp("b128", [128, 1], f32, isOutput=False)
    ins["onesG"] = dp("onesG", [128, 3 * 32], bf16, isOutput=False)
    ins["esel"] = dp("esel", [128, 2], bf16, isOutput=False)
    ins["bsel"] = dp("bsel", [128, J * 128], bf16, isOutput=False)
    ins["rotsel"] = dp("rotsel", [128, 128], bf16, isOutput=False)
    ins["rotred"] = dp("rotred", [128, 128], bf16, isOutput=False)
    ins["rsel"] = dp("rsel", [64, 2 * 128], bf16, isOutput=False)
    ins["texp"] = dp("texp", [128, 2 * 128], bf16, isOutput=False)
    ins["cm1"] = dp("cm1", [128, 1], f32, isOutput=False)
    ins["ident"] = dp("ident", [128, 128], bf16, isOutput=False)
    ins["cmask"] = dp("cmask", [128, CH], bf16, isOutput=False)
    ins["seG"] = dp("seG", [128, 128], bf16, isOutput=False)
    outs = [dp(f"outT{s}", [128, N], bf16, isOutput=True) for s in range(2)]

    with tile.TileContext(nc) as tc:
        with (
            nc.allow_low_precision(
                reason="bf16 weighted-value accumulation; rel-tol 2e-2"),
            tc.tile_pool(name="consts", bufs=1) as cpool,
            tc.tile_pool(name="data", bufs=1) as dpool,
            tc.tile_pool(name="work", bufs=2) as wpool,
            tc.tile_pool(name="ecp", bufs=6) as ecpool,
            tc.tile_pool(name="prods", bufs=24) as ppool,
            tc.tile_pool(name="bcast", bufs=6) as bpool,
            tc.tile_pool(name="tmpp", bufs=3) as tpool,
            tc.tile_pool(name="mmp", bufs=8) as mmpool,
            tc.tile_pool(name="thp", bufs=1) as thpool,
            tc.tile_pool(name="psA", bufs=1, space="PSUM") as psA,   # scps
            tc.tile_pool(name="psB", bufs=2, space="PSUM") as psB,   # e-bcast
            tc.tile_pool(name="psACC", bufs=2, space="PSUM") as psACC,  # acc
            tc.tile_pool(name="psC", bufs=1, space="PSUM") as psC,   # rot seq
        ):
            # DMA order tuned for ramp: chunk-0 score path first.
            qT = [None, None]
            kTp = [None, None]
            vTp = [None, None]
            vsh = [None, None]
            c_pbc = [None, None]
            qT[0] = dpool.tile([128, N], bf16, tag="qT0", name="qT0")
            nc.sync.dma_start(out=qT[0][:, 0:CH], in_=ins["qT0"][:, 0:CH])
            kTp[0] = dpool.tile([128, NP_], bf16, tag="kTp0", name="kTp0")
            nc.gpsimd.memset(kTp[0][:, 0:PAD], 0.0)
            nc.sync.dma_start(out=kTp[0][:, PAD:PAD + CH],
                              in_=ins["kTp0"][:, 0:CH])
            c_onesG = cpool.tile([128, 3 * 32], bf16, tag="c_onesG")
            nc.sync.dma_start(out=c_onesG, in_=ins["onesG"][:])
            c_seG = cpool.tile([128, 128], bf16, tag="c_seG")
            nc.sync.dma_start(out=c_seG, in_=ins["seG"][:])
            c_pbc[0] = cpool.tile([128, 1], f32, tag="c_pbc0", name="c_pbc0")
            nc.sync.dma_start(out=c_pbc[0], in_=ins["pbc0"][:])
            c_cmask = cpool.tile([128, CH], bf16, tag="c_cmask")
            nc.sync.dma_start(out=c_cmask, in_=ins["cmask"][:])
            nc.sync.dma_start(out=qT[0][:, CH:], in_=ins["qT0"][:, CH:])
            nc.sync.dma_start(out=kTp[0][:, PAD + CH:],
                              in_=ins["kTp0"][:, CH:])
            qT[1] = dpool.tile([128, N], bf16, tag="qT1", name="qT1")
            nc.sync.dma_start(out=qT[1], in_=ins["qT1"][:])
            kTp[1] = dpool.tile([128, NP_], bf16, tag="kTp1", name="kTp1")
            nc.gpsimd.memset(kTp[1][:, 0:PAD], 0.0)
            nc.sync.dma_start(out=kTp[1][:, PAD:], in_=ins["kTp1"][:])
            c_pbc[1] = cpool.tile([128, 1], f32, tag="c_pbc1", name="c_pbc1")
            nc.sync.dma_start(out=c_pbc[1], in_=ins["pbc1"][:])
            c_esel = cpool.tile([128, 2], bf16, tag="c_esel")
            nc.sync.dma_start(out=c_esel, in_=ins["esel"][:])
            c_bsel = cpool.tile([128, J * 128], bf16, tag="c_bsel")
            nc.sync.dma_start(out=c_bsel, in_=ins["bsel"][:])
            c_ident = cpool.tile([128, 128], bf16, tag="c_ident")
            nc.sync.dma_start(out=c_ident, in_=ins["ident"][:])
            vTp[0] = dpool.tile([128, NP_], bf16, tag="vTp0", name="vTp0")
            nc.gpsimd.memset(vTp[0][:, 0:PAD], 0.0)
            nc.sync.dma_start(out=vTp[0][:, PAD:], in_=ins["vTp0"][:])
            vTp[1] = dpool.tile([128, NP_], bf16, tag="vTp1", name="vTp1")
            nc.gpsimd.memset(vTp[1][:, 0:PAD], 0.0)
            nc.sync.dma_start(out=vTp[1][:, PAD:], in_=ins["vTp1"][:])
            y128 = dpool.tile([128, N], bf16, tag="y128")
            nc.sync.dma_start(out=y128, in_=ins["y128"][:])
            z128 = dpool.tile([128, N], bf16, tag="z128")
            nc.sync.dma_start(out=z128, in_=ins["z128"][:])
            c_g128 = cpool.tile([128, 1], f32, tag="c_g128")
            nc.sync.dma_start(out=c_g128, in_=ins["g128"][:])
            c_b128 = cpool.tile([128, 1], f32, tag="c_b128")
            nc.sync.dma_start(out=c_b128, in_=ins["b128"][:])
            vsh[0] = dpool.tile([128, N], bf16, tag="vsh0", name="vsh0")
            nc.sync.dma_start(out=vsh[0], in_=ins["vsh0"][:])
            c_rotsel = cpool.tile([128, 128], bf16, tag="c_rotsel")
            nc.sync.dma_start(out=c_rotsel, in_=ins["rotsel"][:])
            c_rotred = cpool.tile([128, 128], bf16, tag="c_rotred")
            nc.sync.dma_start(out=c_rotred, in_=ins["rotred"][:])
            c_rsel = cpool.tile([64, 2 * 128], bf16, tag="c_rsel")
            nc.sync.dma_start(out=c_rsel, in_=ins["rsel"][:])
            c_texp = cpool.tile([128, 2 * 128], bf16, tag="c_texp")
            nc.sync.dma_start(out=c_texp, in_=ins["texp"][:])
            c_cm1 = cpool.tile([128, 1], f32, tag="c_cm1")
            nc.sync.dma_start(out=c_cm1, in_=ins["cm1"][:])
            vsh[1] = dpool.tile([128, N], bf16, tag="vsh1", name="vsh1")
            nc.sync.dma_start(out=vsh[1], in_=ins["vsh1"][:])

            # ---------- trig pre-pass: trig_full for BOTH s ----------
            trig_full = dpool.tile([128, N], bf16, tag="trig_full")
            for pc in range(NCHUNK):
                n0 = pc * CH
                ub = wpool.tile([128, CH], bf16, tag="ub")
                nc.vector.tensor_mul(ub, y128[:, n0:n0 + CH],
                                     z128[:, n0:n0 + CH])
                th = thpool.tile([128, CH], f32, tag="th")
                nc.vector.tensor_scalar(
                    out=th, in0=ub,
                    scalar1=c_g128[:, 0:1], scalar2=c_b128[:, 0:1],
                    op0=AluOp.mult, op1=AluOp.add,
                )
                nc.vector.add_range_wrap(th, th, 0.0, np.pi, 2.0 * np.pi)
                trg = wpool.tile([128, CH], bf16, tag="trg")
                nc.scalar.activation(out=trg, in_=th,
                                     func=Act.Sin, bias=0.0, scale=1.0)
                nc.vector.tensor_scalar_add(trig_full[:, n0:n0 + CH],
                                            trg, c_cm1[:, 0:1])

            def phase_A(c):
                n0 = c * CH
                # ---------- [A] scores + exp (per s) ----------
                nprod = POOL_NPROD_RAMP if c == 0 else POOL_NPROD
                ec = [None, None]
                for s in range(2):
                    ec[s] = ecpool.tile([128, CH], bf16, tag="ec",
                                        name=f"ec{s}")
                    for hf in range(2):
                        h0 = hf * CHA
                        scps = psA.tile([128, CHA], f32, tag="scps")
                        for i, d in enumerate(OFFSETS):
                            a, g = i % 4, i // 4
                            eng = (nc.gpsimd if (2 * i + hf) % 24 < nprod
                                   else nc.vector)
                            pr = ppool.tile([128, CHA], bf16, tag="pr")
                            eng.tensor_mul(
                                pr,
                                kTp[s][:, PAD - d + n0 + h0:
                                       PAD - d + n0 + h0 + CHA],
                                qT[s][:, n0 + h0:n0 + h0 + CHA],
                            )
                            nc.tensor.matmul(
                                out=scps[32 * a:32 * a + 32, :],
                                lhsT=c_onesG[:, 32 * g:32 * g + 32],
                                rhs=pr,
                                start=(g == 0), stop=False,
                                tile_position=(0, 32 * a),
                            )
                        nc.tensor.matmul(
                            out=scps,
                            lhsT=c_seG,
                            rhs=qT[s][:, n0 + h0:n0 + h0 + CHA],
                            start=False, stop=True,
                        )
                        nc.scalar.activation(
                            out=ec[s][:, h0:h0 + CHA], in_=scps,
                            func=Act.Exp, bias=c_pbc[s][:, 0:1], scale=SC,
                        )
                    if c == 0:
                        nc.vector.tensor_mul(ec[s], ec[s], c_cmask)

                # ---------- denominators (both s share recip) ----------
                rcb = wpool.tile([34, CH], bf16, tag="rcb")
                for hf in range(2):
                    h0 = hf * CHA
                    denps = psC.tile([128, CHA], f32, tag="rps", name="denps")
                    for s in range(2):
                        nc.tensor.matmul(
                            out=denps[32 * s:32 * s + 2, :],
                            lhsT=c_esel,
                            rhs=ec[s][:, h0:h0 + CHA],
                            start=True, stop=True,
                        )
                    rf = tpool.tile([34, CHA], f32, tag="rf")
                    nc.vector.tensor_scalar_add(rf, denps[0:34, :], 1e-30)
                    nc.vector.reciprocal(rcb[:, h0:h0 + CHA], rf)
                return ec, rcb

            def phase_B(c, ec, rcb):
                n0 = c * CH
                for s in range(2):
                    # ---------- [D] e-broadcast + weighted v ----------
                    mms = []
                    for i, d in enumerate(OFFSETS):
                        bps = psB.tile([128, CH], f32, tag="bps")
                        for hf in range(2):
                            h0 = hf * CHA
                            nc.tensor.matmul(
                                out=bps[:, h0:h0 + CHA],
                                lhsT=c_bsel[:, i * 128:i * 128 + 128],
                                rhs=ec[s][:, h0:h0 + CHA],
                                start=True, stop=True,
                            )
                        vsl = vTp[s][:, PAD - d + n0: PAD - d + n0 + CH]
                        mm = mmpool.tile([128, CH], bf16, tag="mm")
                        if i in ACT_COPY_D:
                            bcb = bpool.tile([128, CH], bf16, tag="bcb")
                            nc.scalar.activation(out=bcb, in_=bps,
                                                 func=Act.Copy, bias=0.0,
                                                 scale=1.0)
                            nc.vector.tensor_mul(mm, bcb, vsl)
                        else:
                            nc.vector.tensor_mul(mm, bps, vsl)
                        mms.append(mm)

                    outcb = wpool.tile([128, CH], bf16, tag="outcb")
                    accs = [psACC.tile([128, CHA], f32, tag="accps",
                                       name=f"accps{hh}") for hh in range(2)]
                    # interleave both halves' ident-accumulates per offset so
                    # each mm tile frees right after its pair of matmuls
                    for i, mm in enumerate(mms):
                        for hf in range(2):
                            h0 = hf * CHA
                            nc.tensor.matmul(
                                out=accs[hf],
                                lhsT=c_ident,
                                rhs=mm[:, h0:h0 + CHA],
                                start=(i == 0), stop=False,
                            )
                    for hf in range(2):
                        h0 = hf * CHA
                        accps = accs[hf]
                        # ---------- [R] rotation correction (into accps) --
                        t8ps = psC.tile([128, CHA], f32, tag="rps",
                                        name="t8ps")
                        nc.tensor.matmul(
                            out=t8ps,
                            lhsT=c_texp[:, s * 128:s * 128 + 128],
                            rhs=trig_full[:, n0 + h0:n0 + h0 + CHA],
                            start=True, stop=True,
                        )
                        vss = vsh[s][:, n0 + h0:n0 + h0 + CHA]
                        prot = tpool.tile([128, CHA], bf16, tag="prot")
                        nc.vector.tensor_mul(prot, t8ps, vss)
                        erps = psC.tile([128, CHA], f32, tag="rps",
                                        name="erps")
                        nc.tensor.matmul(
                            out=erps,
                            lhsT=c_rotsel,
                            rhs=ec[s][:, h0:h0 + CHA],
                            start=True, stop=True,
                        )
                        vful = tpool.tile([128, CHA], bf16, tag="vful")
                        nc.vector.tensor_mul(vful, erps, prot)
                        nc.tensor.matmul(
                            out=accps,
                            lhsT=c_rotred,
                            rhs=vful,
                            start=False, stop=True,
                        )
                        # ---------- [E] normalize ----------
                        rbps = psC.tile([128, CHA], f32, tag="rps",
                                        name="rbps")
                        nc.tensor.matmul(
                            out=rbps,
                            lhsT=c_rsel[32 * s:32 * s + 2,
                                        s * 128:s * 128 + 128],
                            rhs=rcb[32 * s:32 * s + 2, h0:h0 + CHA],
                            start=True, stop=True,
                        )
                        rbb = bpool.tile([128, CHA], bf16, tag="rbb")
                        nc.scalar.activation(out=rbb, in_=rbps,
                                             func=Act.Copy, bias=0.0,
                                             scale=1.0)
                        nc.vector.tensor_mul(outcb[:, h0:h0 + CHA],
                                             accps, rbb)
                    nc.sync.dma_start(out=outs[s][:, n0:n0 + CH], in_=outcb)

            for c in range(NCHUNK):
                ec_c, rcb_c = phase_A(c)
                phase_B(c, ec_c, rcb_c)

    nc.compile()
    return nc


def get_program():
    global _PROGRAM
    if _PROGRAM is None:
        _PROGRAM = _build_program()
    return _PROGRAM


def _shift_np(x, d):
    """out[n] = x[n-d], zeros for n < d; shift along axis 0."""
    out = np.zeros_like(x)
    out[d:] = x[:-d] if d > 0 else x
    return out


def _bf16(x):
    import ml_dtypes
    return np.asarray(x, dtype=np.float32).astype(ml_dtypes.bfloat16)


def _shared_consts(se_global):
    c = {}
    onesG = np.zeros((128, 3 * 32), np.float32)
    for g in range(3):
        for bh in range(2):
            onesG[64 * bh:64 * bh + 64, 32 * g + 2 * g + bh] = 1.0
    c["onesG"] = _bf16(onesG)
    esel = np.zeros((128, 2), np.float32)
    for i in range(J):
        for bh in range(2):
            esel[_rsc(i, bh), bh] = 1.0
    c["esel"] = _bf16(esel)
    bsel = np.zeros((128, J * 128), np.float32)
    for i in range(J):
        for bh in range(2):
            for hd in range(64):
                bsel[_rsc(i, bh), i * 128 + 64 * bh + hd] = 1.0
    c["bsel"] = _bf16(bsel)
    rotsel = np.zeros((128, 128), np.float32)
    for bh in range(2):
        for i8 in range(8):
            abs_i = i8 + 4
            for t in range(8):
                rotsel[_rsc(abs_i, bh), _rrot(bh, i8, t)] = 1.0
    c["rotsel"] = _bf16(rotsel)
    rotred = np.zeros((128, 128), np.float32)
    sign_map = {0: ((0, 1.0), (3, -1.0)), 1: ((1, 1.0), (2, 1.0)),
                2: ((4, 1.0), (7, -1.0)), 3: ((5, 1.0), (6, 1.0))}
    for bh in range(2):
        for ch in range(4):
            col = bh * 64 + ch
            for i8 in range(8):
                for t, sgn in sign_map[ch]:
                    rotred[_rrot(bh, i8, t), col] = sgn
    c["rotred"] = _bf16(rotred)
    rsel = np.zeros((64, 2 * 128), np.float32)
    for s in range(2):
        for bh in range(2):
            rsel[32 * s + bh, s * 128 + 64 * bh: s * 128 + 64 * bh + 64] = 1.0
    c["rsel"] = _bf16(rsel)
    texp = np.zeros((128, 2 * 128), np.float32)
    for s in range(2):
        for bh in range(2):
            for i8 in range(8):
                for p in range(2):
                    for cs in range(2):
                        src = _rtc(s, bh, i8, p, cs)
                        for c2 in range(2):
                            t = 4 * p + 2 * cs + c2
                            texp[src, s * 128 + _rrot(bh, i8, t)] = 1.0
    c["texp"] = _bf16(texp)
    cm1 = np.zeros((128, 1), np.float32)
    for s in range(2):
        for bh in range(2):
            for i8 in range(8):
                for p in range(2):
                    cm1[_rtc(s, bh, i8, p, 0), 0] = -1.0
    c["cm1"] = cm1
    c["ident"] = _bf16(np.eye(128, dtype=np.float32))
    seG = np.zeros((128, 128), np.float32)
    for i in range(J):
        for bh in range(2):
            for hd in range(64):
                seG[64 * bh + hd, _rsc(i, bh)] = se_global[i, hd]
    c["seG"] = _bf16(seG)
    cmask = np.ones((128, CH), np.float32)
    for i, d in enumerate(OFFSETS):
        r = _rsc(i, 0)
        cmask[r:r + 2, 0:d] = 0.0
    c["cmask"] = _bf16(cmask)
    return c


def _core_inputs(core, q, k, v, pb, se, phase_base, phase_gain, y_pre, z_pre,
                 shared):
    m = dict(shared)
    y128 = np.zeros((128, N), np.float32)
    z128 = np.zeros((128, N), np.float32)
    g128 = np.zeros((128, 1), np.float32)
    b128 = np.zeros((128, 1), np.float32)
    for s in range(2):
        bhs = [4 * core + 2 * s, 4 * core + 2 * s + 1]
        qT = np.zeros((128, N), np.float32)
        kTp = np.zeros((128, N), np.float32)
        vTp = np.zeros((128, N), np.float32)
        vsh = np.zeros((128, N), np.float32)
        pbc = np.zeros((128, 1), np.float32)
        for lbh, bh in enumerate(bhs):
            b, h = bh // H, bh % H
            r0 = lbh * 64
            qT[r0:r0 + 64, :] = q[b, h].T
            kTp[r0:r0 + 64, :] = k[b, h].T
            vTp[r0:r0 + 64, :] = v[b, h].T
            for i8, d in enumerate(ROT):
                for t in range(8):
                    r = _rrot(lbh, i8, t)
                    vsh[r, :] = _shift_np(
                        np.asarray(v[b, h, :, T_CH[t]]), d)
                for p in range(2):
                    for cs in range(2):
                        r = _rtc(s, lbh, i8, p, cs)
                        y128[r, :] = y_pre[b, h, :, p]
                        z128[r, :] = _shift_np(
                            np.asarray(z_pre[b, h, :, p]), d)
                        g128[r, 0] = phase_gain[i8, h, p]
                        b128[r, 0] = phase_base[i8, h, p] + (
                            np.pi / 2.0 if cs == 0 else 0.0)
            for i in range(J):
                pbc[_rsc(i, lbh), 0] = pb[i, h]
        m[f"qT{s}"] = _bf16(qT)
        m[f"kTp{s}"] = _bf16(kTp)
        m[f"vTp{s}"] = _bf16(vTp)
        m[f"vsh{s}"] = _bf16(vsh)
        m[f"pbc{s}"] = pbc
    m["y128"] = _bf16(y128)
    m["z128"] = _bf16(z128)
    m["g128"] = g128
    m["b128"] = b128
    return m


def make_in_maps(q, k, v, pb, se, phase_base, phase_gain, y_pre, z_pre):
    shared = _shared_consts(np.asarray(se))
    args = [np.asarray(x) for x in
            (q, k, v, pb, se, phase_base, phase_gain, y_pre, z_pre)]
    return [_core_inputs(c, *args, shared) for c in range(NCORES)]


def assemble_output(results):
    out = np.zeros((B, H, N, HD), np.float32)
    for core in range(NCORES):
        for s in range(2):
            outT = np.asarray(results[core][f"outT{s}"], dtype=np.float32)
            for lbh in range(2):
                bh = 4 * core + 2 * s + lbh
                b, h = bh // H, bh % H
                out[b, h] = outT[lbh * 64:(lbh + 1) * 64, :].T
    return out


def kernel(**inputs):
    from concourse.bass_utils import run_bass_kernel_spmd

    nc = get_program()
    in_maps = make_in_maps(**inputs)
    res = run_bass_kernel_spmd(nc, in_maps, core_ids=list(range(NCORES)))
    return assemble_output(res.results)


if __name__ == "__main__":
    get_program()
    print("program built + compiled OK")
